# revision 49
# baseline (speedup 1.0000x reference)
"""AnchorSet2NodeMPNN Trainium2 kernel (8 NeuronCores, graph-parallel).

Each core handles one graph (N=384 nodes, A=64 anchors, H=256, E=64).

v2: fp8 DoubleRow main loop + PE-fused anchor-sum  (~282us vs 427us v1).
  Pair columns are ordered a-major (col = 8*a + n within a chunk) so the
  L3 matmul can anchor-sum in PSUM via a stride-0 output AP (same psum
  word revisited every 8 columns; back-to-back same-address accumulation
  is broken on HW, 8-apart is fine).
  d^2[n,a] = |nx|^2 + |ax|^2 - 2 nx.ax           (rank-5 K=5 matmul)
  t'[n,a]  = exp(0.5*ln(relu(d^2)*s))            (ACT-only; avoids the
    sqrt table so the whole kernel uses ONE activation table set --
    see the get_activation_tables override above; staged to DRAM
    chunk-major so rbf gathers are contiguous broadcasts)
  Layer 1 factored: pair @ W1a = nf@W1a[:H] + af@W1a[H:2H] + rbf@W1a[2H:]
    folded into fp8 DoubleRow matmuls: psum = na.T@ind_n (+) [w1r;AF'].T@[rbf;ind_a]
  Layer 2 per-pair fp8 DoubleRow (K=512 as 2 instrs of 2 k-tiles).
  Layer 3 fused with the anchor-mean on the PE: upd psum accumulates all
    64 anchor columns at the same address (stride-0 out AP, start=False
    onto a zeroed region; the zeroing matmul stays on the PE queue).
  LayerNorms + node MLP run feature-transposed (LN stats via gpsimd
  partition allreduce, rstd = exp(-0.5*ln(v))); final PE-transpose back.

Weights are host-prepacked (fp8e4 / bf16) so no on-device weight casts.
fp8 DoubleRow = 2x PE throughput (2 k-tiles/instr at 1 col/cycle); the
cost model's 0.5 cyc/row is optimistic.  Extending fp8 to the small node
MLP triggered chip activity-throttling and was reverted.
fp8 everywhere in the per-pair path gives ~3.2e-3 relative error.
"""
import numpy as np
import ml_dtypes

import concourse.bass as bass
import concourse.mybir as mybir
import concourse.tile as tile
from concourse import bacc
from concourse.bass_utils import run_bass_kernel_spmd

# All activations used here (exp/ln/relu/square/identity/copy) live in the
# "natural_log_exp_and_others" table set, but the table-load pass assigns
# each function its first-containing set, thrashing ACT_TABLE_LOADs (1.3us
# each) between exp_and_others and natural_log_exp_and_others. Pin every
# activation to the one covering set so exactly one load is emitted.
_orig_get_tables = bacc.get_activation_tables
def _single_table_set(arch):
    tabs = _orig_get_tables(arch)
    keep = "natural_log_exp_and_others"
    return {k: (v if k == keep else set()) for k, v in tabs.items()}
bacc.get_activation_tables = _single_table_set

F32 = mybir.dt.float32
BF16 = mybir.dt.bfloat16
F8 = mybir.dt.float8e4
AF = mybir.ActivationFunctionType
DR = mybir.MatmulPerfMode.DoubleRow
F8NP = ml_dtypes.float8_e4m3
BFNP = ml_dtypes.bfloat16

B, N, A, H, E = 8, 384, 64, 256, 64
RBF_D_MAX = 20.0
SIGMA = RBF_D_MAX / E                    # 0.3125
MU = np.linspace(0.0, RBF_D_MAX, E).astype(np.float32)
NC_CHUNKS = 48                           # chunks of 8 nodes x 64 anchors
D2_SCALE = 0.01 / SIGMA**2               # t' = sqrt(d2 * D2_SCALE) = (d/10)/sigma

MAST_COLS = 8192
SLOT0 = MAST_COLS                        # rbf slot region start in rbm
NSLOT = 6


def _consts():
    # a-major pair order: col j = 8*a + n (n in 0..7, a in 0..63)
    # anchor indicator, two 512-col chunk halves per slot
    mast_a = np.zeros((64, 1024), np.float32)
    for j in range(1024):
        mast_a[(j % 512) // 8, j] = 1.0
    # node indicator: mast_n[k, 4096q+512s+j] = 1 iff k == 64q+8s+(j%8)
    mast_n = np.zeros((128, 8192), np.float32)
    for q in range(2):
        for s in range(8):
            for j in range(512):
                mast_n[64 * q + 8 * s + j % 8, 4096 * q + 512 * s + j] = 1.0
    ident = np.eye(128, dtype=np.float32)
    pairm = np.zeros((128, 128), np.float32)
    for j in range(128):
        for m in range(128):
            if j % 64 == m % 64:
                pairm[j, m] = 1.0
    negmusig = np.tile(-(MU / SIGMA).astype(np.float32), 2)
    ones64 = np.ones((1, 64), np.float32)
    return dict(
        c_mast_a=mast_a.astype(F8NP),
        c_mast_n=mast_n.astype(F8NP),
        c_ident=ident,
        c_pairm=pairm,
        c_negmusig=negmusig,
        c_ones64=ones64.astype(BFNP),
    )


def _pack_lnrows(inputs):
    """[128, 512] f32: col 128k+p, row j = v[128*(j//64)+p] for the
    transposed-layout LN affine (k: ln1_g, ln1_b, ln2_g, ln2_b)."""
    out = np.empty((128, 512), np.float32)
    for k, nm in enumerate(("ln1_g", "ln1_b", "ln2_g", "ln2_b")):
        v = np.asarray(inputs[nm], np.float32)
        out[0:64, 128 * k:128 * k + 128] = v[None, 0:128]
        out[64:128, 128 * k:128 * k + 128] = v[None, 128:256]
    return np.ascontiguousarray(out)


def _pack_biases(inputs):
    """Column-pack per-feature vectors: [128, 24] f32, layout-only."""
    cols = []
    for k, n in (("b1b", 4), ("b1c", 2), ("b2a", 4), ("b2b", 4), ("b2c", 2),
                 ("ln1_g", 2), ("ln1_b", 2), ("ln2_g", 2), ("ln2_b", 2)):
        v = np.asarray(inputs[k], np.float32)
        cols.append(v.reshape(n, 128).T)
    return np.ascontiguousarray(np.concatenate(cols, axis=1))


def _kpack(w, kt, dt):
    """[K, F] -> [128, kt*F] with col = F*kc + f, row p = k % 128."""
    K, F = w.shape
    assert K == 128 * kt
    out = np.empty((128, kt * F), np.float32)
    for kc in range(kt):
        out[:, F * kc:F * kc + F] = w[128 * kc:128 * kc + 128, :]
    return np.ascontiguousarray(out).astype(dt)


def _pack_weights(inputs):
    W1a = np.asarray(inputs["W1a"], np.float32)
    d = dict(
        w1nf_bf=_kpack(W1a[0:256], 2, BFNP),       # [128,1024]
        w1af_bf=_kpack(W1a[256:512], 2, BFNP),     # [128,1024]
        w1r8=np.ascontiguousarray(W1a[512:576]).astype(F8NP),   # [64,512]
        w1b8=_kpack(np.asarray(inputs["W1b"], np.float32), 4, F8NP),  # [128,2048]
        w1c8=_kpack(np.asarray(inputs["W1c"], np.float32), 4, F8NP),  # [128,1024]
        w2a_bf=_kpack(np.asarray(inputs["W2a"], np.float32), 2, BFNP),
        w2b_bf=_kpack(np.asarray(inputs["W2b"], np.float32), 4, BFNP),
        w2c_bf=_kpack(np.asarray(inputs["W2c"], np.float32), 4, BFNP),
        b1a_bf=np.asarray(inputs["b1a"], np.float32).reshape(1, 512).astype(BFNP),
    )
    return d


def _build():
    nc = bacc.Bacc("TRN2", target_bir_lowering=False, debug=False)

    # ---- parameters ----
    p_nx = nc.declare_dram_parameter("node_x", [N, 3], F32, isOutput=False)
    p_ax = nc.declare_dram_parameter("anchor_x", [A, 3], F32, isOutput=False)
    p_nf = nc.declare_dram_parameter("node_features", [N, H], F32, isOutput=False)
    p_af = nc.declare_dram_parameter("anchor_features", [A, H], F32, isOutput=False)
    p_mask = nc.declare_dram_parameter("node_mask", [N], F32, isOutput=False)
    p_w1nf = nc.declare_dram_parameter("w1nf_bf", [128, 1024], BF16, isOutput=False)
    p_w1af = nc.declare_dram_parameter("w1af_bf", [128, 1024], BF16, isOutput=False)
    p_w1r8 = nc.declare_dram_parameter("w1r8", [64, 512], F8, isOutput=False)
    p_w1b8 = nc.declare_dram_parameter("w1b8", [128, 2048], F8, isOutput=False)
    p_w1c8 = nc.declare_dram_parameter("w1c8", [128, 1024], F8, isOutput=False)
    p_w2a = nc.declare_dram_parameter("w2a_bf", [128, 1024], BF16, isOutput=False)
    p_w2b = nc.declare_dram_parameter("w2b_bf", [128, 2048], BF16, isOutput=False)
    p_w2c = nc.declare_dram_parameter("w2c_bf", [128, 1024], BF16, isOutput=False)
    p_b1a = nc.declare_dram_parameter("b1a_bf", [1, 512], BF16, isOutput=False)
    c_mast_a = nc.declare_dram_parameter("c_mast_a", [64, 1024], F8, isOutput=False)
    c_mast_n = nc.declare_dram_parameter("c_mast_n", [128, 8192], F8, isOutput=False)
    c_biases = nc.declare_dram_parameter("c_biases", [128, 24], F32, isOutput=False)
    c_lnrows = nc.declare_dram_parameter("c_lnrows", [128, 512], F32, isOutput=False)
    c_ident = nc.declare_dram_parameter("c_ident", [128, 128], F32, isOutput=False)
    c_pairm = nc.declare_dram_parameter("c_pairm", [128, 128], F32, isOutput=False)
    c_negmusig = nc.declare_dram_parameter("c_negmusig", [2 * E], F32, isOutput=False)
    c_ones64 = nc.declare_dram_parameter("c_ones64", [1, 64], BF16, isOutput=False)
    p_out = nc.declare_dram_parameter("out", [N, H], F32, isOutput=True)

    # t staged chunk-major: element (c, a, n) at 512c + 8a + n (a-major pairs)
    t_dram = [nc.dram_tensor(f"t_scratch{r}", [16, 512], F32) for r in range(3)]

    with tile.TileContext(nc) as tc:
        with (
            tc.tile_pool(name="wp", bufs=1) as wp,
            tc.tile_pool(name="psA", bufs=2, space="PSUM") as psA,
            tc.tile_pool(name="psB", bufs=3, space="PSUM") as psB,
            tc.tile_pool(name="psC", bufs=1, space="PSUM") as psC,
            tc.tile_pool(name="tbp", bufs=3) as tbp,
            tc.tile_pool(name="qp", bufs=3) as qp,
            tc.tile_pool(name="h1p", bufs=3) as h1p,
            tc.tile_pool(name="h2p", bufs=3) as h2p,
            tc.tile_pool(name="outp", bufs=2) as outp,
        ):
            dma = nc.sync.dma_start

            # ================= phase 0: loads =================
            nx_sb = [wp.tile([128, 3], F32, tag=f"nx{r}", name=f"nx{r}")
                     for r in range(3)]
            for r in range(3):
                dma(nx_sb[r], p_nx[128 * r:128 * r + 128, :])
            ax_sb = wp.tile([64, 3], F32)
            dma(ax_sb, p_ax[:])
            ident = wp.tile([128, 128], F32); dma(ident, c_ident[:])
            pairm = wp.tile([128, 128], F32)
            nc.scalar.dma_start(pairm, c_pairm[:])
            negmu = wp.tile([128, 1], F32)
            dma(negmu, c_negmusig[:].rearrange("(p o) -> p o", o=1))
            w1nf = wp.tile([128, 1024], BF16); dma(w1nf, p_w1nf[:])
            w1af = wp.tile([128, 1024], BF16); dma(w1af, p_w1af[:])
            nf_sb = [wp.tile([128, 256], F32, tag=f"nfsb{r}", name=f"nfsb{r}")
                     for r in range(3)]
            for r in range(3):
                nc.scalar.dma_start(nf_sb[r], p_nf[128 * r:128 * r + 128, :])

            # one big fp8 tile: node masters + rbf slots (cross-region DR APs)
            rbm = wp.tile([128, MAST_COLS + NSLOT * 1024], F8, name="rbm")
            nc.gpsimd.dma_start(rbm[:, 0:4096], c_mast_n[:, 0:4096])
            nc.gpsimd.dma_start(rbm[:, 4096:8192], c_mast_n[:, 4096:8192])
            for i in range(NSLOT):
                nc.scalar.dma_start(
                    rbm[64:128, SLOT0 + 1024 * i:SLOT0 + 1024 * i + 1024],
                    c_mast_a[:])

            # L1 lhsT tile: na (3 r-blocks) then w1raf
            l1w = wp.tile([128, 2048], F8, name="l1w")
            dma(l1w[0:64, 1536:2048], p_w1r8[:])

            w1b8 = wp.tile([128, 2048], F8)
            nc.gpsimd.dma_start(w1b8, p_w1b8[:])
            w1c8 = wp.tile([128, 1024], F8)
            nc.gpsimd.dma_start(w1c8, p_w1c8[:])
            b1a_rb = wp.tile([1, 512], BF16); dma(b1a_rb, p_b1a[:])
            ones64 = wp.tile([1, 64], BF16); dma(ones64, c_ones64[:])
            af_sb = wp.tile([64, 256], F32)
            nc.scalar.dma_start(af_sb, p_af[:])

            # packed bias columns
            bias_pack = wp.tile([128, 24], F32)
            dma(bias_pack, c_biases[:])
            lnrows = wp.tile([128, 512], F32)
            nc.scalar.dma_start(lnrows, c_lnrows[:])
            off = [0]
            def bp(n):
                t = bias_pack[:, off[0]:off[0] + n]
                off[0] += n
                return t
            b1b_c = bp(4); b1c_c = bp(2); b2a_c = bp(4); b2b_c = bp(4)
            b2c_c = bp(2); ln1g_c = bp(2); ln1b_c = bp(2); ln2g_c = bp(2)
            ln2b_c = bp(2)
            eps_c = wp.tile([128, 1], F32)
            nc.vector.memset(eps_c, 1e-5)
            mask_b = wp.tile([128, N], F32)
            mb_src = p_mask[0:1]
            nc.sync.dma_start(
                out=mask_b,
                in_=bass.AP(tensor=mb_src.tensor, offset=0, ap=[[0, 128], [1, N]]),
            )
            zeros_b = wp.tile([128, 512], BF16)
            nc.vector.memset(zeros_b, 0.0)
            zw = wp.tile([128, 128], F8)
            nc.vector.memset(zw, 0.0)

            # ================= phase 0: geometry (defs) =================
            axs = wp.tile([64, 3], F32)
            nc.vector.tensor_tensor(axs, ax_sb, ax_sb, op=mybir.AluOpType.mult)
            aa2 = wp.tile([64, 1], F32)
            nc.vector.reduce_sum(aa2, axs, axis=mybir.AxisListType.X)
            aug_a = wp.tile([64, 5], F32)
            nc.vector.tensor_scalar_mul(aug_a[:, 0:3], ax_sb, -2.0)
            nc.vector.memset(aug_a[:, 3:4], 1.0)
            nc.vector.tensor_copy(aug_a[:, 4:5], aa2)
            p_t = psA.tile([128, 64], F32, tag="a")
            nc.tensor.transpose(p_t[0:5, 0:64], aug_a, ident[0:64, 0:64])
            axaug = wp.tile([5, 64], F32)
            nc.vector.tensor_copy(axaug, p_t[0:5, 0:64])

            nfT_b = wp.tile([128, 768], BF16)   # nf.T bf16, kc-major
            nfT_f = wp.tile([128, 768], F32)    # nf.T f32

            def geom_block(r):
                """distances for node block r -> t_dram[r]; nf.T transposes."""
                nxs = wp.tile([128, 3], F32, tag="nxs")
                nc.vector.tensor_tensor(nxs, nx_sb[r], nx_sb[r], op=mybir.AluOpType.mult)
                nn2 = wp.tile([128, 1], F32, tag="nn2")
                nc.vector.reduce_sum(nn2, nxs, axis=mybir.AxisListType.X)
                aug_n = wp.tile([128, 5], F32, tag="augn")
                nc.vector.tensor_copy(aug_n[:, 0:3], nx_sb[r])
                nc.vector.tensor_copy(aug_n[:, 3:4], nn2)
                nc.vector.memset(aug_n[:, 4:5], 1.0)
                p_tn = psB.tile([128, 128], F32, tag="b")
                nc.tensor.transpose(p_tn[0:5, :], aug_n, ident)
                nxaugT = wp.tile([5, 128], F32, tag="nxaugT")
                nc.vector.tensor_copy(nxaugT, p_tn[0:5, :])
                p_d2 = psA.tile([64, 128], F32, tag="a")
                nc.tensor.matmul(p_d2, axaug, nxaugT, start=True, stop=True)
                # t = sqrt(relu(d2)*s) as exp(0.5*ln(.)): ACT-only chain on
                # the single loaded table set (relu clamps fp slop, ln(0)=-inf
                # exponentiates to t=0, matching the d2<=0 limit)
                d2c = wp.tile([64, 128], F32, tag="d2c")
                nc.scalar.activation(d2c, p_d2, AF.Relu, bias=0.0, scale=D2_SCALE)
                tl = wp.tile([64, 128], F32, tag="tl")
                nc.scalar.activation(tl, d2c, AF.Ln, bias=0.0, scale=1.0)
                t_sb = wp.tile([64, 128], F32, tag="tsb")
                nc.scalar.activation(t_sb, tl, AF.Exp, bias=0.0, scale=0.5)
                dma(
                    out=bass.AP(tensor=t_dram[r][:].tensor, offset=0,
                                ap=[[8, 64], [512, 16], [1, 8]]),
                    in_=t_sb.rearrange("p (c n) -> p c n", c=16, n=8),
                )
                for c in range(2):
                    p_tr = psB.tile([128, 128], F32, tag="b")
                    nc.tensor.transpose(p_tr, nf_sb[r][:, 128 * c:128 * c + 128], ident)
                    nc.vector.tensor_copy(
                        nfT_b[:, 384 * c + 128 * r:384 * c + 128 * r + 128], p_tr)
                    nc.vector.tensor_copy(
                        nfT_f[:, 384 * c + 128 * r:384 * c + 128 * r + 128], p_tr)

            def na_block(r):
                """NA matmuls -> fp8 into l1w na region r."""
                p_na = psB.tile([128, 512], F32, tag="b")
                for kc in range(2):
                    nc.tensor.matmul(
                        p_na,
                        nfT_b[:, 384 * kc + 128 * r:384 * kc + 128 * r + 128],
                        w1nf[:, 512 * kc:512 * kc + 512],
                        start=(kc == 0), stop=(kc == 1),
                    )
                with nc.allow_low_precision(reason="fp8 main-loop operands"):
                    nc.vector.tensor_copy(l1w[:, 512 * r:512 * r + 512], p_na)

            def afp_chain():
                """af.T; AF' = af@W1a[H:2H] + b1a -> fp8 l1w rows 64:128."""
                afT_b = wp.tile([128, 128], BF16)
                for c in range(2):
                    p_tr = psA.tile([128, 64], F32, tag="a")
                    nc.tensor.transpose(p_tr[:, 0:64], af_sb[:, 128 * c:128 * c + 128],
                                        ident[0:64, 0:64])
                    nc.vector.tensor_copy(afT_b[:, 64 * c:64 * c + 64], p_tr[:, 0:64])
                p_af2 = psB.tile([64, 512], F32, tag="b")
                for kc in range(2):
                    nc.tensor.matmul(p_af2, afT_b[:, 64 * kc:64 * kc + 64],
                                     w1af[:, 512 * kc:512 * kc + 512],
                                     start=(kc == 0), stop=False)
                nc.tensor.matmul(p_af2, ones64, b1a_rb, start=False, stop=True)
                with nc.allow_low_precision(reason="fp8 main-loop operands"):
                    nc.vector.tensor_copy(l1w[64:128, 1536:2048], p_af2)

            updT = [wp.tile([128, 256], F32, tag=f"updT{r}", name=f"updT{r}")
                    for r in range(3)]

            def ap3(t, offset, d1, n1, d2_, n2):
                return bass.AP(tensor=t.tensor, offset=t.offset + offset,
                               ap=[list(t.ap[0]), [d1, n1], [d2_, n2]])

            # ================= main loop (software-pipelined) =================
            def stageA(pp):
                """t gather + rbf for superchunk pair (2pp, 2pp+1).

                tb rows 0:64 = sc 2pp pairs (a-major), rows 64:128 = sc 2pp+1;
                Exp writes fp8 straight into rbm slots (rows 0:64)."""
                tb = tbp.tile([128, 1024], F32, tag="tb", name=f"tb{pp}")
                for h in range(2):
                    sc = 2 * pp + h
                    cl = 2 * sc - 16 * (sc // 8)
                    nc.sync.dma_start(
                        out=tb[64 * h:64 * h + 64, :],
                        in_=bass.AP(tensor=t_dram[sc // 8][:].tensor,
                                    offset=512 * cl,
                                    ap=[[0, 64], [1, 1024]]),
                    )
                qx = qp.tile([128, 1024], F32, tag="qx", name=f"qx{pp}")
                nc.scalar.activation(qx, tb, AF.Square, bias=negmu[:, 0:1], scale=1.0)
                for h in range(2):
                    slot = (2 * pp + h) % NSLOT
                    nc.scalar.activation(
                        rbm[0:64, SLOT0 + 1024 * slot:SLOT0 + 1024 * slot + 1024],
                        qx[64 * h:64 * h + 64, :], AF.Exp, bias=0.0, scale=-1.0)

            def stageB(c):
                """L1 fp8 DoubleRow matmuls + relu -> h1 (fp8)."""
                q = (c // 8) % 2
                s = c % 8
                r = c // 16
                mast_off = 4096 * q + 512 * s
                rt_off = SLOT0 + 1024 * ((c // 2) % NSLOT) + 512 * (c % 2)
                rhs = ap3(rbm, mast_off, rt_off - mast_off, 2, 1, 512)
                h1 = h1p.tile([128, 2048], F8, tag="h1", name=f"h1_{c}")
                for hh in range(2):
                    p1 = psA.tile([128, 1024], F32, tag="a", name=f"p1_{c}_{hh}")
                    for i in range(2):
                        fc = 2 * hh + i
                        lhsT = ap3(l1w, 512 * r + 128 * fc, 1536 - 512 * r, 2, 1, 128)
                        nc.tensor.matmul(p1[:, 512 * i:512 * i + 512], lhsT, rhs,
                                         start=True, stop=True, perf_mode=DR)
                    with nc.allow_low_precision(reason="fp8 main-loop operands"):
                        nc.scalar.activation(h1[:, 1024 * hh:1024 * hh + 1024],
                                             p1, AF.Relu, bias=0.0, scale=1.0)
                return h1

            def stageC(c, h1):
                """L2 fp8 DR + relu+bias -> h2s (fp8); L3 fp8 DR with PE
                anchor-sum (stride-0 psum accumulate) -> updT."""
                h2s = h2p.tile([128, 2048], F8, tag="h2", name=f"h2_{c}")
                for fc in range(4):
                    p2 = psB.tile([128, 512], F32, tag="b", name=f"p2_{c}_{fc}")
                    for kp in range(2):
                        lhsT = ap3(w1b8, 1024 * kp + 128 * fc, 512, 2, 1, 128)
                        rhs = ap3(h1, 1024 * kp, 512, 2, 1, 512)
                        nc.tensor.matmul(p2, lhsT, rhs, start=(kp == 0),
                                         stop=(kp == 1), perf_mode=DR)
                    with nc.allow_low_precision(reason="fp8 main-loop operands"):
                        nc.vector.scalar_tensor_tensor(
                            h2s[:, 512 * fc:512 * fc + 512], p2,
                            b1b_c[:, fc:fc + 1], zeros_b,
                            op0=mybir.AluOpType.add, op1=mybir.AluOpType.max)
                # L3 + anchor-sum: U[fo*8+n] += sum_a sum_k w1c[k,fo]h2[k,8a+n]
                U = psB.tile([128, 16], F32, tag="b", name=f"U_{c}")
                nc.tensor.matmul(U, zw, zw[:, 0:16], start=True, stop=True)
                for fo in range(2):
                    out2 = bass.AP(tensor=U.tensor, offset=U.offset + 8 * fo,
                                   ap=[list(U.ap[0]), [0, 64], [1, 8]])
                    for kp in range(2):
                        lhsT = ap3(w1c8, 512 * kp + 128 * fo, 256, 2, 1, 128)
                        rhs = ap3(h2s, 1024 * kp, 512, 2, 1, 512)
                        nc.tensor.matmul(out2, lhsT, rhs, start=False,
                                         stop=(fo == 1 and kp == 1),
                                         perf_mode=DR, skip_group_check=True)
                r = c // 16
                dst = bass.AP(tensor=updT[r].tensor,
                              offset=updT[r].offset + 8 * (c % 16),
                              ap=[list(updT[r].ap[0]), [128, 2], [1, 8]])
                src = bass.AP(tensor=U.tensor, offset=U.offset,
                              ap=[list(U.ap[0]), [8, 2], [1, 8]])
                nc.vector.tensor_copy(dst, src)

            # ====== phase 2 (node path), per-128-node block, overlapped ======
            zeros_f = wp.tile([128, 128], F32)
            nc.vector.memset(zeros_f, 0.0)
            zeros_b8 = wp.tile([128, 128], BF16)
            nc.vector.memset(zeros_b8, 0.0)
            p2w = {}

            def load_phase2_weights():
                for nm, prm, shp in (("w2a", p_w2a, [128, 1024]),
                                     ("w2b", p_w2b, [128, 2048]),
                                     ("w2c", p_w2c, [128, 1024])):
                    t = wp.tile(shp, BF16, name=nm, tag=nm)
                    dma(t, prm[:])
                    p2w[nm] = t

            def block_ln_h(bid, w, x_in, g_c, b_c, out_t):
                """LN over 256 feats for w nodes; x_in/out_t [128, 2w]
                compact fo-major (col = w*fo + n)."""
                x3 = x_in.rearrange("p (f n) -> p f n", f=2)
                red = wp.tile([128, 2 * w], F32, tag=f"lnr{bid}", name=f"lnr{bid}")
                nc.gpsimd.partition_all_reduce(
                    red, x_in, channels=128, reduce_op=bass.bass_isa.ReduceOp.add)
                Ssum = wp.tile([128, w], F32, tag=f"lnS{bid}", name=f"lnS{bid}")
                nc.vector.tensor_tensor(Ssum, red[:, 0:w], red[:, w:2 * w],
                                        op=mybir.AluOpType.add)
                Sb = bass.AP(tensor=Ssum.tensor, offset=Ssum.offset,
                             ap=[list(Ssum.ap[0]), [0, 2], list(Ssum.ap[1])])
                xc = wp.tile([128, 2 * w], F32, tag=f"lnxc{bid}", name=f"lnxc{bid}")
                nc.vector.scalar_tensor_tensor(
                    xc.rearrange("p (f n) -> p f n", f=2), Sb, -1.0 / 256.0, x3,
                    op0=mybir.AluOpType.mult, op1=mybir.AluOpType.add)
                sq = wp.tile([128, 2 * w], F32, tag=f"lnsq{bid}", name=f"lnsq{bid}")
                nc.vector.tensor_tensor(sq, xc, xc, op=mybir.AluOpType.mult)
                red2 = wp.tile([128, 2 * w], F32, tag=f"lnr2{bid}", name=f"lnr2{bid}")
                nc.gpsimd.partition_all_reduce(
                    red2, sq, channels=128, reduce_op=bass.bass_isa.ReduceOp.add)
                V = wp.tile([128, w], F32, tag=f"lnV{bid}", name=f"lnV{bid}")
                nc.vector.tensor_tensor(V, red2[:, 0:w], red2[:, w:2 * w],
                                        op=mybir.AluOpType.add)
                sd = wp.tile([128, w], F32, tag=f"lnsd{bid}", name=f"lnsd{bid}")
                nc.scalar.activation(sd, V, AF.Ln, bias=eps_c[:, 0:1],
                                     scale=1.0 / 256.0)
                rstd = wp.tile([128, w], F32, tag=f"lnrstd{bid}", name=f"lnrstd{bid}")
                nc.scalar.activation(rstd, sd, AF.Exp, bias=0.0, scale=-0.5)
                rb = bass.AP(tensor=rstd.tensor, offset=rstd.offset,
                             ap=[list(rstd.ap[0]), [0, 2], list(rstd.ap[1])])
                y = wp.tile([128, 2 * w], F32, tag=f"lny{bid}", name=f"lny{bid}")
                nc.vector.tensor_tensor(y.rearrange("p (f n) -> p f n", f=2),
                                        xc.rearrange("p (f n) -> p f n", f=2), rb,
                                        op=mybir.AluOpType.mult)
                for fo in range(2):
                    nc.scalar.activation(out_t[:, w * fo:w * fo + w],
                                         y[:, w * fo:w * fo + w],
                                         AF.Identity, bias=b_c[:, fo:fo + 1],
                                         scale=g_c[:, fo:fo + 1])

            p2state = {}

            def phase2_blk(bid, r, hf, w, piece, late=False):
                """phase 2 on a w-node block (nodes 128r + w*hf ..+w);
                intermediate tiles compact (col = w*fo + n or w*fc + n)."""
                no = 128 * r + w * hf
                psP = psB if late else psC
                ptag = "b" if late else "c"
                st = p2state.setdefault(bid, {})
                mb = mask_b[:, no:no + w]
                mb2 = bass.AP(tensor=mb.tensor, offset=mb.offset,
                              ap=[list(mb.ap[0]), [0, 2], list(mb.ap[1])])
                if piece == 0:
                    # upd = updT/64 + b1c; x1 + LN1 (+ bf16 cast)
                    upd = wp.tile([128, 2 * w], F32, tag=f"upd{bid}", name=f"upd{bid}")
                    for fo in range(2):
                        nc.scalar.activation(
                            upd[:, w * fo:w * fo + w],
                            updT[r][:, 128 * fo + w * hf:128 * fo + w * hf + w],
                            AF.Identity, bias=b1c_c[:, fo:fo + 1], scale=1.0 / 64.0)
                    x1 = wp.tile([128, 2 * w], F32, tag=f"x1{bid}", name=f"x1{bid}")
                    tmpw = wp.tile([128, 2 * w], F32, tag=f"updm{bid}", name=f"updm{bid}")
                    nc.vector.tensor_tensor(
                        tmpw.rearrange("p (f n) -> p f n", f=2),
                        upd.rearrange("p (f n) -> p f n", f=2), mb2,
                        op=mybir.AluOpType.mult)
                    nfs = nfT_f[:, no:no + w]
                    nfv = bass.AP(tensor=nfs.tensor, offset=nfs.offset,
                                  ap=[list(nfs.ap[0]), [384, 2], list(nfs.ap[1])])
                    nc.vector.tensor_tensor(
                        x1.rearrange("p (f n) -> p f n", f=2),
                        tmpw.rearrange("p (f n) -> p f n", f=2), nfv,
                        op=mybir.AluOpType.add)
                    nf1 = wp.tile([128, 2 * w], F32, tag=f"nf1{bid}", name=f"nf1{bid}")
                    block_ln_h(bid, w, x1, ln1g_c, ln1b_c, nf1)
                    nf1_b = wp.tile([128, 2 * w], BF16, tag=f"nf1b{bid}",
                                    name=f"nf1b{bid}")
                    nc.vector.tensor_copy(nf1_b, nf1)
                    st["nf1"], st["nf1_b"] = nf1, nf1_b
                elif piece == 1:
                    nf1_b = st["nf1_b"]
                    w2a_b = p2w["w2a"]
                    g1 = wp.tile([128, 4 * w], BF16, tag=f"g1{bid}", name=f"g1{bid}")
                    for fc in range(4):
                        p = psP.tile([128, w], F32, tag=ptag, name=f"pg1_{bid}_{fc}")
                        for kc in range(2):
                            nc.tensor.matmul(
                                p, w2a_b[:, 512 * kc + 128 * fc:512 * kc + 128 * fc + 128],
                                nf1_b[:, w * kc:w * kc + w],
                                start=(kc == 0), stop=(kc == 1))
                        nc.vector.scalar_tensor_tensor(
                            g1[:, w * fc:w * fc + w], p, b2a_c[:, fc:fc + 1],
                            zeros_b8[:, 0:w], op0=mybir.AluOpType.add,
                            op1=mybir.AluOpType.max)
                    st["g1"] = g1
                elif piece == 2:
                    g1 = st["g1"]
                    w2b_b = p2w["w2b"]
                    g2 = wp.tile([128, 4 * w], BF16, tag=f"g2{bid}", name=f"g2{bid}")
                    for fc in range(4):
                        p = psP.tile([128, w], F32, tag=ptag, name=f"pg2_{bid}_{fc}")
                        for kc in range(4):
                            nc.tensor.matmul(
                                p, w2b_b[:, 512 * kc + 128 * fc:512 * kc + 128 * fc + 128],
                                g1[:, w * kc:w * kc + w],
                                start=(kc == 0), stop=(kc == 3))
                        nc.vector.scalar_tensor_tensor(
                            g2[:, w * fc:w * fc + w], p, b2b_c[:, fc:fc + 1],
                            zeros_b8[:, 0:w], op0=mybir.AluOpType.add,
                            op1=mybir.AluOpType.max)
                    st["g2"] = g2
                elif piece == 3:
                    g2, nf1 = st["g2"], st["nf1"]
                    w2c_b = p2w["w2c"]
                    upd2 = wp.tile([128, 2 * w], F32, tag=f"upd2{bid}",
                                   name=f"upd2{bid}")
                    for fo in range(2):
                        p = psP.tile([128, w], F32, tag=ptag, name=f"pu2_{bid}_{fo}")
                        for kc in range(4):
                            nc.tensor.matmul(
                                p, w2c_b[:, 256 * kc + 128 * fo:256 * kc + 128 * fo + 128],
                                g2[:, w * kc:w * kc + w],
                                start=(kc == 0), stop=(kc == 3))
                        nc.vector.scalar_tensor_tensor(
                            upd2[:, w * fo:w * fo + w], p, b2c_c[:, fo:fo + 1],
                            zeros_f[:, 0:w], op0=mybir.AluOpType.add,
                            op1=mybir.AluOpType.add)
                    x2 = wp.tile([128, 2 * w], F32, tag=f"x2{bid}", name=f"x2{bid}")
                    tmpw2 = wp.tile([128, 2 * w], F32, tag=f"updm2{bid}",
                                    name=f"updm2{bid}")
                    nc.vector.tensor_tensor(
                        tmpw2.rearrange("p (f n) -> p f n", f=2),
                        upd2.rearrange("p (f n) -> p f n", f=2), mb2,
                        op=mybir.AluOpType.mult)
                    nc.vector.tensor_tensor(x2, tmpw2, nf1, op=mybir.AluOpType.add)
                    outT = wp.tile([128, 2 * w], F32, tag=f"outT{bid}",
                                   name=f"outT{bid}")
                    block_ln_h(bid, w, x2, ln2g_c, ln2b_c, outT)
                    st["outT"] = outT
                else:
                    outT = st["outT"]
                    o_sb = outp.tile([w, 256], F32, tag="osb", name=f"osb{bid}")
                    for fo in range(2):
                        p_tr = psP.tile([w, 128], F32, tag=ptag,
                                        name=f"ptr_{bid}_{fo}")
                        nc.tensor.transpose(p_tr, outT[:, w * fo:w * fo + w], ident)
                        nc.vector.tensor_copy(o_sb[:, 128 * fo:128 * fo + 128], p_tr)
                    nc.sync.dma_start(p_out[no:no + w, :], o_sb)

            # full 128-node blocks for r=0,1 (fewer mid-loop ops); the last
            # block is split into 64-node halves so only half trails the loop
            P2BLOCKS = (("A0", 0, 0, 128, 18), ("A1", 1, 0, 128, 34),
                        ("H4", 2, 0, 64, 42), ("H5", 2, 1, 64, 50))

            # phase 0 emission, r=0-first so the pipeline starts early
            geom_block(0)
            afp_chain()
            na_block(0)
            stageA(0)
            geom_block(1)
            na_block(1)
            stageA(1)
            geom_block(2)
            na_block(2)
            h1_live = {}
            for c in range(NC_CHUNKS + 1):
                if c == 2:
                    load_phase2_weights()
                if c % 4 == 0 and c // 4 + 2 < 12:
                    stageA(c // 4 + 2)
                if c < NC_CHUNKS:
                    h1_live[c] = stageB(c)
                if 0 <= c - 1 < NC_CHUNKS:
                    stageC(c - 1, h1_live.pop(c - 1))
                for bid, r, hf, w, base in P2BLOCKS:
                    if base <= c < base + 10 and (c - base) % 2 == 0 and c < 48:
                        phase2_blk(bid, r, hf, w, (c - base) // 2)
                if c == 48:
                    for bid, r, hf, w, base in P2BLOCKS:
                        for k in range(5):
                            if base + 2 * k >= 48:
                                phase2_blk(bid, r, hf, w, k, late=True)

    nc.compile()
    return nc


_NC = None
_CONSTS = _consts()


def build_in_maps(inputs):
    shared = dict(_CONSTS)
    shared.update(_pack_weights(inputs))
    shared["c_biases"] = _pack_biases(inputs)
    shared["c_lnrows"] = _pack_lnrows(inputs)

    node_x = np.asarray(inputs["node_x"], np.float32)
    anchor_x = np.asarray(inputs["anchor_x"], np.float32)
    node_features = np.asarray(inputs["node_features"], np.float32)
    anchor_features = np.asarray(inputs["anchor_features"], np.float32)
    node_mask = np.asarray(inputs["node_mask"], np.float32)

    in_maps = []
    for b in range(B):
        m = dict(shared)
        m["node_x"] = np.ascontiguousarray(node_x[b * N:(b + 1) * N])
        m["anchor_x"] = np.ascontiguousarray(anchor_x[b * A:(b + 1) * A])
        m["node_features"] = np.ascontiguousarray(node_features[b * N:(b + 1) * N])
        m["anchor_features"] = np.ascontiguousarray(
            anchor_features[b * A:(b + 1) * A])
        m["node_mask"] = np.ascontiguousarray(node_mask[b * N:(b + 1) * N])
        in_maps.append(m)
    return in_maps


def kernel(**inputs):
    global _NC
    if _NC is None:
        _NC = _build()
    in_maps = build_in_maps(inputs)
    res = run_bass_kernel_spmd(_NC, in_maps, core_ids=list(range(B)))
    return np.concatenate([res.results[b]["out"] for b in range(B)], axis=0)


# revision 50
# speedup vs baseline: 1.0079x; 1.0079x over previous
"""AnchorSet2NodeMPNN Trainium2 kernel (8 NeuronCores, graph-parallel).

Each core handles one graph (N=384 nodes, A=64 anchors, H=256, E=64).

v2: fp8 DoubleRow main loop + PE-fused anchor-sum  (~282us vs 427us v1).
  Pair columns are ordered a-major (col = 8*a + n within a chunk) so the
  L3 matmul can anchor-sum in PSUM via a stride-0 output AP (same psum
  word revisited every 8 columns; back-to-back same-address accumulation
  is broken on HW, 8-apart is fine).
  d^2[n,a] = |nx|^2 + |ax|^2 - 2 nx.ax           (rank-5 K=5 matmul)
  t'[n,a]  = exp(0.5*ln(relu(d^2)*s))            (ACT-only; avoids the
    sqrt table so the whole kernel uses ONE activation table set --
    see the get_activation_tables override above; staged to DRAM
    chunk-major so rbf gathers are contiguous broadcasts)
  Layer 1 factored: pair @ W1a = nf@W1a[:H] + af@W1a[H:2H] + rbf@W1a[2H:]
    folded into fp8 DoubleRow matmuls: psum = na.T@ind_n (+) [w1r;AF'].T@[rbf;ind_a]
  Layer 2 per-pair fp8 DoubleRow (K=512 as 2 instrs of 2 k-tiles).
  Layer 3 fused with the anchor-mean on the PE: upd psum accumulates all
    64 anchor columns at the same address (stride-0 out AP, start=False
    onto a zeroed region; the zeroing matmul stays on the PE queue).
  LayerNorms + node MLP run feature-transposed (LN stats via gpsimd
  partition allreduce, rstd = exp(-0.5*ln(v))); final PE-transpose back.

Weights are host-prepacked (fp8e4 / bf16) so no on-device weight casts.
fp8 DoubleRow = 2x PE throughput (2 k-tiles/instr at 1 col/cycle); the
cost model's 0.5 cyc/row is optimistic.  Extending fp8 to the small node
MLP triggered chip activity-throttling and was reverted.
fp8 everywhere in the per-pair path gives ~3.2e-3 relative error.
"""
import numpy as np
import ml_dtypes

import concourse.bass as bass
import concourse.mybir as mybir
import concourse.tile as tile
from concourse import bacc
from concourse.bass_utils import run_bass_kernel_spmd

# All activations used here (exp/ln/relu/square/identity/copy) live in the
# "natural_log_exp_and_others" table set, but the table-load pass assigns
# each function its first-containing set, thrashing ACT_TABLE_LOADs (1.3us
# each) between exp_and_others and natural_log_exp_and_others. Pin every
# activation to the one covering set so exactly one load is emitted.
_orig_get_tables = bacc.get_activation_tables
def _single_table_set(arch):
    tabs = _orig_get_tables(arch)
    keep = "natural_log_exp_and_others"
    return {k: (v if k == keep else set()) for k, v in tabs.items()}
bacc.get_activation_tables = _single_table_set

F32 = mybir.dt.float32
BF16 = mybir.dt.bfloat16
F8 = mybir.dt.float8e4
AF = mybir.ActivationFunctionType
DR = mybir.MatmulPerfMode.DoubleRow
F8NP = ml_dtypes.float8_e4m3
BFNP = ml_dtypes.bfloat16

B, N, A, H, E = 8, 384, 64, 256, 64
RBF_D_MAX = 20.0
SIGMA = RBF_D_MAX / E                    # 0.3125
MU = np.linspace(0.0, RBF_D_MAX, E).astype(np.float32)
NC_CHUNKS = 48                           # chunks of 8 nodes x 64 anchors
D2_SCALE = 0.01 / SIGMA**2               # t' = sqrt(d2 * D2_SCALE) = (d/10)/sigma

MAST_COLS = 8192
SLOT0 = MAST_COLS                        # rbf slot region start in rbm
NSLOT = 6


def _consts():
    # a-major pair order: col j = 8*a + n (n in 0..7, a in 0..63)
    # anchor indicator, two 512-col chunk halves per slot
    mast_a = np.zeros((64, 1024), np.float32)
    for j in range(1024):
        mast_a[(j % 512) // 8, j] = 1.0
    # node indicator: mast_n[k, 4096q+512s+j] = 1 iff k == 64q+8s+(j%8)
    mast_n = np.zeros((128, 8192), np.float32)
    for q in range(2):
        for s in range(8):
            for j in range(512):
                mast_n[64 * q + 8 * s + j % 8, 4096 * q + 512 * s + j] = 1.0
    ident = np.eye(128, dtype=np.float32)
    pairm = np.zeros((128, 128), np.float32)
    for j in range(128):
        for m in range(128):
            if j % 64 == m % 64:
                pairm[j, m] = 1.0
    negmusig = np.tile(-(MU / SIGMA).astype(np.float32), 2)
    ones64 = np.ones((1, 64), np.float32)
    return dict(
        c_mast_a=mast_a.astype(F8NP),
        c_mast_n=mast_n.astype(F8NP),
        c_ident=ident,
        c_pairm=pairm,
        c_negmusig=negmusig,
        c_ones64=ones64.astype(BFNP),
    )


def _pack_lnrows(inputs):
    """[128, 512] f32: col 128k+p, row j = v[128*(j//64)+p] for the
    transposed-layout LN affine (k: ln1_g, ln1_b, ln2_g, ln2_b)."""
    out = np.empty((128, 512), np.float32)
    for k, nm in enumerate(("ln1_g", "ln1_b", "ln2_g", "ln2_b")):
        v = np.asarray(inputs[nm], np.float32)
        out[0:64, 128 * k:128 * k + 128] = v[None, 0:128]
        out[64:128, 128 * k:128 * k + 128] = v[None, 128:256]
    return np.ascontiguousarray(out)


def _pack_biases(inputs):
    """Column-pack per-feature vectors: [128, 24] f32, layout-only."""
    cols = []
    for k, n in (("b1b", 4), ("b1c", 2), ("b2a", 4), ("b2b", 4), ("b2c", 2),
                 ("ln1_g", 2), ("ln1_b", 2), ("ln2_g", 2), ("ln2_b", 2)):
        v = np.asarray(inputs[k], np.float32)
        cols.append(v.reshape(n, 128).T)
    return np.ascontiguousarray(np.concatenate(cols, axis=1))


def _kpack(w, kt, dt):
    """[K, F] -> [128, kt*F] with col = F*kc + f, row p = k % 128."""
    K, F = w.shape
    assert K == 128 * kt
    out = np.empty((128, kt * F), np.float32)
    for kc in range(kt):
        out[:, F * kc:F * kc + F] = w[128 * kc:128 * kc + 128, :]
    return np.ascontiguousarray(out).astype(dt)


def _pack_weights(inputs):
    W1a = np.asarray(inputs["W1a"], np.float32)
    d = dict(
        w1nf_bf=_kpack(W1a[0:256], 2, BFNP),       # [128,1024]
        w1af_bf=_kpack(W1a[256:512], 2, BFNP),     # [128,1024]
        w1r8=np.ascontiguousarray(W1a[512:576]).astype(F8NP),   # [64,512]
        w1b8=_kpack(np.asarray(inputs["W1b"], np.float32), 4, F8NP),  # [128,2048]
        w1c8=_kpack(np.asarray(inputs["W1c"], np.float32), 4, F8NP),  # [128,1024]
        w2a_bf=_kpack(np.asarray(inputs["W2a"], np.float32), 2, BFNP),
        w2b_bf=_kpack(np.asarray(inputs["W2b"], np.float32), 4, BFNP),
        w2c_bf=_kpack(np.asarray(inputs["W2c"], np.float32), 4, BFNP),
        b1a_bf=np.asarray(inputs["b1a"], np.float32).reshape(1, 512).astype(BFNP),
    )
    return d


def _build():
    nc = bacc.Bacc("TRN2", target_bir_lowering=False, debug=False)

    # ---- parameters ----
    p_nx = nc.declare_dram_parameter("node_x", [N, 3], F32, isOutput=False)
    p_ax = nc.declare_dram_parameter("anchor_x", [A, 3], F32, isOutput=False)
    p_nf = nc.declare_dram_parameter("node_features", [N, H], F32, isOutput=False)
    p_af = nc.declare_dram_parameter("anchor_features", [A, H], F32, isOutput=False)
    p_mask = nc.declare_dram_parameter("node_mask", [N], F32, isOutput=False)
    p_w1nf = nc.declare_dram_parameter("w1nf_bf", [128, 1024], BF16, isOutput=False)
    p_w1af = nc.declare_dram_parameter("w1af_bf", [128, 1024], BF16, isOutput=False)
    p_w1r8 = nc.declare_dram_parameter("w1r8", [64, 512], F8, isOutput=False)
    p_w1b8 = nc.declare_dram_parameter("w1b8", [128, 2048], F8, isOutput=False)
    p_w1c8 = nc.declare_dram_parameter("w1c8", [128, 1024], F8, isOutput=False)
    p_w2a = nc.declare_dram_parameter("w2a_bf", [128, 1024], BF16, isOutput=False)
    p_w2b = nc.declare_dram_parameter("w2b_bf", [128, 2048], BF16, isOutput=False)
    p_w2c = nc.declare_dram_parameter("w2c_bf", [128, 1024], BF16, isOutput=False)
    p_b1a = nc.declare_dram_parameter("b1a_bf", [1, 512], BF16, isOutput=False)
    c_mast_a = nc.declare_dram_parameter("c_mast_a", [64, 1024], F8, isOutput=False)
    c_mast_n = nc.declare_dram_parameter("c_mast_n", [128, 8192], F8, isOutput=False)
    c_biases = nc.declare_dram_parameter("c_biases", [128, 24], F32, isOutput=False)
    c_lnrows = nc.declare_dram_parameter("c_lnrows", [128, 512], F32, isOutput=False)
    c_ident = nc.declare_dram_parameter("c_ident", [128, 128], F32, isOutput=False)
    c_pairm = nc.declare_dram_parameter("c_pairm", [128, 128], F32, isOutput=False)
    c_negmusig = nc.declare_dram_parameter("c_negmusig", [2 * E], F32, isOutput=False)
    c_ones64 = nc.declare_dram_parameter("c_ones64", [1, 64], BF16, isOutput=False)
    p_out = nc.declare_dram_parameter("out", [N, H], F32, isOutput=True)

    # t staged chunk-major: element (c, a, n) at 512c + 8a + n (a-major pairs)
    t_dram = [nc.dram_tensor(f"t_scratch{r}", [16, 512], F32) for r in range(3)]

    with tile.TileContext(nc) as tc:
        with (
            tc.tile_pool(name="wp", bufs=1) as wp,
            tc.tile_pool(name="psA", bufs=2, space="PSUM") as psA,
            tc.tile_pool(name="psB", bufs=3, space="PSUM") as psB,
            tc.tile_pool(name="psC", bufs=1, space="PSUM") as psC,
            tc.tile_pool(name="tbp", bufs=3) as tbp,
            tc.tile_pool(name="qp", bufs=3) as qp,
            tc.tile_pool(name="h1p", bufs=3) as h1p,
            tc.tile_pool(name="h2p", bufs=3) as h2p,
            tc.tile_pool(name="outp", bufs=2) as outp,
        ):
            dma = nc.sync.dma_start

            # ================= phase 0: loads =================
            nx_sb = [wp.tile([128, 3], F32, tag=f"nx{r}", name=f"nx{r}")
                     for r in range(3)]
            for r in range(3):
                dma(nx_sb[r], p_nx[128 * r:128 * r + 128, :])
            ax_sb = wp.tile([64, 3], F32)
            dma(ax_sb, p_ax[:])
            ident = wp.tile([128, 128], F32); dma(ident, c_ident[:])
            pairm = wp.tile([128, 128], F32)
            nc.scalar.dma_start(pairm, c_pairm[:])
            negmu = wp.tile([128, 1], F32)
            dma(negmu, c_negmusig[:].rearrange("(p o) -> p o", o=1))
            w1nf = wp.tile([128, 1024], BF16); dma(w1nf, p_w1nf[:])
            w1af = wp.tile([128, 1024], BF16); dma(w1af, p_w1af[:])
            nf_sb = [wp.tile([128, 256], F32, tag=f"nfsb{r}", name=f"nfsb{r}")
                     for r in range(3)]
            for r in range(3):
                nc.scalar.dma_start(nf_sb[r], p_nf[128 * r:128 * r + 128, :])

            # one big fp8 tile: node masters + rbf slots (cross-region DR APs)
            rbm = wp.tile([128, MAST_COLS + NSLOT * 1024], F8, name="rbm")
            nc.gpsimd.dma_start(rbm[:, 0:4096], c_mast_n[:, 0:4096])
            nc.gpsimd.dma_start(rbm[:, 4096:8192], c_mast_n[:, 4096:8192])
            for i in range(NSLOT):
                nc.scalar.dma_start(
                    rbm[64:128, SLOT0 + 1024 * i:SLOT0 + 1024 * i + 1024],
                    c_mast_a[:])

            # L1 lhsT tile: na (3 r-blocks) then w1raf
            l1w = wp.tile([128, 2048], F8, name="l1w")
            dma(l1w[0:64, 1536:2048], p_w1r8[:])

            w1b8 = wp.tile([128, 2048], F8)
            nc.gpsimd.dma_start(w1b8, p_w1b8[:])
            w1c8 = wp.tile([128, 1024], F8)
            nc.gpsimd.dma_start(w1c8, p_w1c8[:])
            b1a_rb = wp.tile([1, 512], BF16); dma(b1a_rb, p_b1a[:])
            ones64 = wp.tile([1, 64], BF16); dma(ones64, c_ones64[:])
            af_sb = wp.tile([64, 256], F32)
            nc.scalar.dma_start(af_sb, p_af[:])

            # packed bias columns
            bias_pack = wp.tile([128, 24], F32)
            dma(bias_pack, c_biases[:])
            lnrows = wp.tile([128, 512], F32)
            nc.scalar.dma_start(lnrows, c_lnrows[:])
            off = [0]
            def bp(n):
                t = bias_pack[:, off[0]:off[0] + n]
                off[0] += n
                return t
            b1b_c = bp(4); b1c_c = bp(2); b2a_c = bp(4); b2b_c = bp(4)
            b2c_c = bp(2); ln1g_c = bp(2); ln1b_c = bp(2); ln2g_c = bp(2)
            ln2b_c = bp(2)
            eps_c = wp.tile([128, 1], F32)
            nc.vector.memset(eps_c, 1e-5)
            mask_b = wp.tile([128, N], F32)
            mb_src = p_mask[0:1]
            nc.sync.dma_start(
                out=mask_b,
                in_=bass.AP(tensor=mb_src.tensor, offset=0, ap=[[0, 128], [1, N]]),
            )
            zeros_b = wp.tile([128, 512], BF16)
            nc.vector.memset(zeros_b, 0.0)
            zw = wp.tile([128, 128], F8)
            nc.vector.memset(zw, 0.0)

            # ================= phase 0: geometry (defs) =================
            axs = wp.tile([64, 3], F32)
            nc.vector.tensor_tensor(axs, ax_sb, ax_sb, op=mybir.AluOpType.mult)
            aa2 = wp.tile([64, 1], F32)
            nc.vector.reduce_sum(aa2, axs, axis=mybir.AxisListType.X)
            aug_a = wp.tile([64, 5], F32)
            nc.vector.tensor_scalar_mul(aug_a[:, 0:3], ax_sb, -2.0)
            nc.vector.memset(aug_a[:, 3:4], 1.0)
            nc.vector.tensor_copy(aug_a[:, 4:5], aa2)
            p_t = psA.tile([128, 64], F32, tag="a")
            nc.tensor.transpose(p_t[0:5, 0:64], aug_a, ident[0:64, 0:64])
            axaug = wp.tile([5, 64], F32)
            nc.vector.tensor_copy(axaug, p_t[0:5, 0:64])

            nfT_b = wp.tile([128, 768], BF16)   # nf.T bf16, kc-major
            nfT_f = wp.tile([128, 768], F32)    # nf.T f32

            def geom_block(r):
                """distances for node block r -> t_dram[r]; nf.T transposes."""
                nxs = wp.tile([128, 3], F32, tag="nxs")
                nc.vector.tensor_tensor(nxs, nx_sb[r], nx_sb[r], op=mybir.AluOpType.mult)
                nn2 = wp.tile([128, 1], F32, tag="nn2")
                nc.vector.reduce_sum(nn2, nxs, axis=mybir.AxisListType.X)
                aug_n = wp.tile([128, 5], F32, tag="augn")
                nc.vector.tensor_copy(aug_n[:, 0:3], nx_sb[r])
                nc.vector.tensor_copy(aug_n[:, 3:4], nn2)
                nc.vector.memset(aug_n[:, 4:5], 1.0)
                p_tn = psB.tile([128, 128], F32, tag="b")
                nc.tensor.transpose(p_tn[0:5, :], aug_n, ident)
                nxaugT = wp.tile([5, 128], F32, tag="nxaugT")
                nc.vector.tensor_copy(nxaugT, p_tn[0:5, :])
                p_d2 = psA.tile([64, 128], F32, tag="a")
                nc.tensor.matmul(p_d2, axaug, nxaugT, start=True, stop=True)
                # t = sqrt(relu(d2)*s) as exp(0.5*ln(.)): ACT-only chain on
                # the single loaded table set (relu clamps fp slop, ln(0)=-inf
                # exponentiates to t=0, matching the d2<=0 limit)
                d2c = wp.tile([64, 128], F32, tag="d2c")
                nc.scalar.activation(d2c, p_d2, AF.Relu, bias=0.0, scale=D2_SCALE)
                tl = wp.tile([64, 128], F32, tag="tl")
                nc.scalar.activation(tl, d2c, AF.Ln, bias=0.0, scale=1.0)
                t_sb = wp.tile([64, 128], F32, tag="tsb")
                nc.scalar.activation(t_sb, tl, AF.Exp, bias=0.0, scale=0.5)
                dma(
                    out=bass.AP(tensor=t_dram[r][:].tensor, offset=0,
                                ap=[[8, 64], [512, 16], [1, 8]]),
                    in_=t_sb.rearrange("p (c n) -> p c n", c=16, n=8),
                )
                for c in range(2):
                    p_tr = psB.tile([128, 128], F32, tag="b")
                    nc.tensor.transpose(p_tr, nf_sb[r][:, 128 * c:128 * c + 128], ident)
                    nc.vector.tensor_copy(
                        nfT_b[:, 384 * c + 128 * r:384 * c + 128 * r + 128], p_tr)
                    nc.vector.tensor_copy(
                        nfT_f[:, 384 * c + 128 * r:384 * c + 128 * r + 128], p_tr)

            def na_block(r):
                """NA matmuls -> fp8 into l1w na region r."""
                p_na = psB.tile([128, 512], F32, tag="b")
                for kc in range(2):
                    nc.tensor.matmul(
                        p_na,
                        nfT_b[:, 384 * kc + 128 * r:384 * kc + 128 * r + 128],
                        w1nf[:, 512 * kc:512 * kc + 512],
                        start=(kc == 0), stop=(kc == 1),
                    )
                with nc.allow_low_precision(reason="fp8 main-loop operands"):
                    nc.vector.tensor_copy(l1w[:, 512 * r:512 * r + 512], p_na)

            def afp_chain():
                """af.T; AF' = af@W1a[H:2H] + b1a -> fp8 l1w rows 64:128."""
                afT_b = wp.tile([128, 128], BF16)
                for c in range(2):
                    p_tr = psA.tile([128, 64], F32, tag="a")
                    nc.tensor.transpose(p_tr[:, 0:64], af_sb[:, 128 * c:128 * c + 128],
                                        ident[0:64, 0:64])
                    nc.vector.tensor_copy(afT_b[:, 64 * c:64 * c + 64], p_tr[:, 0:64])
                p_af2 = psB.tile([64, 512], F32, tag="b")
                for kc in range(2):
                    nc.tensor.matmul(p_af2, afT_b[:, 64 * kc:64 * kc + 64],
                                     w1af[:, 512 * kc:512 * kc + 512],
                                     start=(kc == 0), stop=False)
                nc.tensor.matmul(p_af2, ones64, b1a_rb, start=False, stop=True)
                with nc.allow_low_precision(reason="fp8 main-loop operands"):
                    nc.vector.tensor_copy(l1w[64:128, 1536:2048], p_af2)

            updT = [wp.tile([128, 256], F32, tag=f"updT{r}", name=f"updT{r}")
                    for r in range(3)]

            def ap3(t, offset, d1, n1, d2_, n2):
                return bass.AP(tensor=t.tensor, offset=t.offset + offset,
                               ap=[list(t.ap[0]), [d1, n1], [d2_, n2]])

            # ================= main loop (software-pipelined) =================
            def stageA(pp):
                """t gather + rbf for superchunk pair (2pp, 2pp+1).

                tb rows 0:64 = sc 2pp pairs (a-major), rows 64:128 = sc 2pp+1;
                Exp writes fp8 straight into rbm slots (rows 0:64)."""
                tb = tbp.tile([128, 1024], F32, tag="tb", name=f"tb{pp}")
                for h in range(2):
                    sc = 2 * pp + h
                    cl = 2 * sc - 16 * (sc // 8)
                    nc.sync.dma_start(
                        out=tb[64 * h:64 * h + 64, :],
                        in_=bass.AP(tensor=t_dram[sc // 8][:].tensor,
                                    offset=512 * cl,
                                    ap=[[0, 64], [1, 1024]]),
                    )
                qx = qp.tile([128, 1024], F32, tag="qx", name=f"qx{pp}")
                nc.scalar.activation(qx, tb, AF.Square, bias=negmu[:, 0:1], scale=1.0)
                for h in range(2):
                    slot = (2 * pp + h) % NSLOT
                    nc.scalar.activation(
                        rbm[0:64, SLOT0 + 1024 * slot:SLOT0 + 1024 * slot + 1024],
                        qx[64 * h:64 * h + 64, :], AF.Exp, bias=0.0, scale=-1.0)

            def stageB(c):
                """L1 fp8 DoubleRow matmuls + relu -> h1 (fp8)."""
                q = (c // 8) % 2
                s = c % 8
                r = c // 16
                mast_off = 4096 * q + 512 * s
                rt_off = SLOT0 + 1024 * ((c // 2) % NSLOT) + 512 * (c % 2)
                rhs = ap3(rbm, mast_off, rt_off - mast_off, 2, 1, 512)
                h1 = h1p.tile([128, 2048], F8, tag="h1", name=f"h1_{c}")
                for hh in range(2):
                    p1 = psA.tile([128, 1024], F32, tag="a", name=f"p1_{c}_{hh}")
                    for i in range(2):
                        fc = 2 * hh + i
                        lhsT = ap3(l1w, 512 * r + 128 * fc, 1536 - 512 * r, 2, 1, 128)
                        nc.tensor.matmul(p1[:, 512 * i:512 * i + 512], lhsT, rhs,
                                         start=True, stop=True, perf_mode=DR)
                    with nc.allow_low_precision(reason="fp8 main-loop operands"):
                        nc.scalar.activation(h1[:, 1024 * hh:1024 * hh + 1024],
                                             p1, AF.Relu, bias=0.0, scale=1.0)
                return h1

            def stageC(c, h1):
                """L2 fp8 DR + relu+bias -> h2s (fp8); L3 fp8 DR with PE
                anchor-sum (stride-0 psum accumulate) -> updT."""
                h2s = h2p.tile([128, 2048], F8, tag="h2", name=f"h2_{c}")
                for fc in range(4):
                    p2 = psB.tile([128, 512], F32, tag="b", name=f"p2_{c}_{fc}")
                    for kp in range(2):
                        lhsT = ap3(w1b8, 1024 * kp + 128 * fc, 512, 2, 1, 128)
                        rhs = ap3(h1, 1024 * kp, 512, 2, 1, 512)
                        nc.tensor.matmul(p2, lhsT, rhs, start=(kp == 0),
                                         stop=(kp == 1), perf_mode=DR)
                    with nc.allow_low_precision(reason="fp8 main-loop operands"):
                        nc.vector.scalar_tensor_tensor(
                            h2s[:, 512 * fc:512 * fc + 512], p2,
                            b1b_c[:, fc:fc + 1], zeros_b,
                            op0=mybir.AluOpType.add, op1=mybir.AluOpType.max)
                # L3 + anchor-sum: U[fo*8+n] += sum_a sum_k w1c[k,fo]h2[k,8a+n]
                U = psB.tile([128, 16], F32, tag="b", name=f"U_{c}")
                nc.tensor.matmul(U, zw, zw[:, 0:16], start=True, stop=True)
                for fo in range(2):
                    out2 = bass.AP(tensor=U.tensor, offset=U.offset + 8 * fo,
                                   ap=[list(U.ap[0]), [0, 64], [1, 8]])
                    for kp in range(2):
                        lhsT = ap3(w1c8, 512 * kp + 128 * fo, 256, 2, 1, 128)
                        rhs = ap3(h2s, 1024 * kp, 512, 2, 1, 512)
                        nc.tensor.matmul(out2, lhsT, rhs, start=False,
                                         stop=(fo == 1 and kp == 1),
                                         perf_mode=DR, skip_group_check=True)
                r = c // 16
                dst = bass.AP(tensor=updT[r].tensor,
                              offset=updT[r].offset + 8 * (c % 16),
                              ap=[list(updT[r].ap[0]), [128, 2], [1, 8]])
                src = bass.AP(tensor=U.tensor, offset=U.offset,
                              ap=[list(U.ap[0]), [8, 2], [1, 8]])
                nc.vector.tensor_copy(dst, src)

            # ====== phase 2 (node path), per-128-node block, overlapped ======
            zeros_f = wp.tile([128, 128], F32)
            nc.vector.memset(zeros_f, 0.0)
            zeros_b8 = wp.tile([128, 128], BF16)
            nc.vector.memset(zeros_b8, 0.0)
            p2w = {}

            def load_phase2_weights():
                for nm, prm, shp in (("w2a", p_w2a, [128, 1024]),
                                     ("w2b", p_w2b, [128, 2048]),
                                     ("w2c", p_w2c, [128, 1024])):
                    t = wp.tile(shp, BF16, name=nm, tag=nm)
                    dma(t, prm[:])
                    p2w[nm] = t

            def block_ln_h(bid, w, x_in, g_c, b_c, out_t):
                """LN over 256 feats for w nodes; x_in/out_t [128, 2w]
                compact fo-major (col = w*fo + n)."""
                x3 = x_in.rearrange("p (f n) -> p f n", f=2)
                red = wp.tile([128, 2 * w], F32, tag=f"lnr{bid}", name=f"lnr{bid}")
                nc.gpsimd.partition_all_reduce(
                    red, x_in, channels=128, reduce_op=bass.bass_isa.ReduceOp.add)
                Ssum = wp.tile([128, w], F32, tag=f"lnS{bid}", name=f"lnS{bid}")
                nc.vector.tensor_tensor(Ssum, red[:, 0:w], red[:, w:2 * w],
                                        op=mybir.AluOpType.add)
                Sb = bass.AP(tensor=Ssum.tensor, offset=Ssum.offset,
                             ap=[list(Ssum.ap[0]), [0, 2], list(Ssum.ap[1])])
                xc = wp.tile([128, 2 * w], F32, tag=f"lnxc{bid}", name=f"lnxc{bid}")
                nc.vector.scalar_tensor_tensor(
                    xc.rearrange("p (f n) -> p f n", f=2), Sb, -1.0 / 256.0, x3,
                    op0=mybir.AluOpType.mult, op1=mybir.AluOpType.add)
                sq = wp.tile([128, 2 * w], F32, tag=f"lnsq{bid}", name=f"lnsq{bid}")
                nc.vector.tensor_tensor(sq, xc, xc, op=mybir.AluOpType.mult)
                red2 = wp.tile([128, 2 * w], F32, tag=f"lnr2{bid}", name=f"lnr2{bid}")
                nc.gpsimd.partition_all_reduce(
                    red2, sq, channels=128, reduce_op=bass.bass_isa.ReduceOp.add)
                V = wp.tile([128, w], F32, tag=f"lnV{bid}", name=f"lnV{bid}")
                nc.vector.tensor_tensor(V, red2[:, 0:w], red2[:, w:2 * w],
                                        op=mybir.AluOpType.add)
                sd = wp.tile([128, w], F32, tag=f"lnsd{bid}", name=f"lnsd{bid}")
                nc.scalar.activation(sd, V, AF.Ln, bias=eps_c[:, 0:1],
                                     scale=1.0 / 256.0)
                rstd = wp.tile([128, w], F32, tag=f"lnrstd{bid}", name=f"lnrstd{bid}")
                nc.scalar.activation(rstd, sd, AF.Exp, bias=0.0, scale=-0.5)
                rb = bass.AP(tensor=rstd.tensor, offset=rstd.offset,
                             ap=[list(rstd.ap[0]), [0, 2], list(rstd.ap[1])])
                y = wp.tile([128, 2 * w], F32, tag=f"lny{bid}", name=f"lny{bid}")
                nc.vector.tensor_tensor(y.rearrange("p (f n) -> p f n", f=2),
                                        xc.rearrange("p (f n) -> p f n", f=2), rb,
                                        op=mybir.AluOpType.mult)
                for fo in range(2):
                    nc.scalar.activation(out_t[:, w * fo:w * fo + w],
                                         y[:, w * fo:w * fo + w],
                                         AF.Identity, bias=b_c[:, fo:fo + 1],
                                         scale=g_c[:, fo:fo + 1])

            p2state = {}

            def phase2_blk(bid, r, hf, w, piece, late=False):
                """phase 2 on a w-node block (nodes 128r + w*hf ..+w);
                intermediate tiles compact (col = w*fo + n or w*fc + n)."""
                no = 128 * r + w * hf
                psP = psB if late else psC
                ptag = "b" if late else "c"
                st = p2state.setdefault(bid, {})
                mb = mask_b[:, no:no + w]
                mb2 = bass.AP(tensor=mb.tensor, offset=mb.offset,
                              ap=[list(mb.ap[0]), [0, 2], list(mb.ap[1])])
                if piece == 0:
                    # upd = updT/64 + b1c; x1 + LN1 (+ bf16 cast)
                    upd = wp.tile([128, 2 * w], F32, tag=f"upd{bid}", name=f"upd{bid}")
                    for fo in range(2):
                        nc.scalar.activation(
                            upd[:, w * fo:w * fo + w],
                            updT[r][:, 128 * fo + w * hf:128 * fo + w * hf + w],
                            AF.Identity, bias=b1c_c[:, fo:fo + 1], scale=1.0 / 64.0)
                    x1 = wp.tile([128, 2 * w], F32, tag=f"x1{bid}", name=f"x1{bid}")
                    tmpw = wp.tile([128, 2 * w], F32, tag=f"updm{bid}", name=f"updm{bid}")
                    nc.vector.tensor_tensor(
                        tmpw.rearrange("p (f n) -> p f n", f=2),
                        upd.rearrange("p (f n) -> p f n", f=2), mb2,
                        op=mybir.AluOpType.mult)
                    nfs = nfT_f[:, no:no + w]
                    nfv = bass.AP(tensor=nfs.tensor, offset=nfs.offset,
                                  ap=[list(nfs.ap[0]), [384, 2], list(nfs.ap[1])])
                    nc.vector.tensor_tensor(
                        x1.rearrange("p (f n) -> p f n", f=2),
                        tmpw.rearrange("p (f n) -> p f n", f=2), nfv,
                        op=mybir.AluOpType.add)
                    nf1 = wp.tile([128, 2 * w], F32, tag=f"nf1{bid}", name=f"nf1{bid}")
                    block_ln_h(bid, w, x1, ln1g_c, ln1b_c, nf1)
                    nf1_b = wp.tile([128, 2 * w], BF16, tag=f"nf1b{bid}",
                                    name=f"nf1b{bid}")
                    nc.vector.tensor_copy(nf1_b, nf1)
                    st["nf1"], st["nf1_b"] = nf1, nf1_b
                elif piece == 1:
                    nf1_b = st["nf1_b"]
                    w2a_b = p2w["w2a"]
                    g1 = wp.tile([128, 4 * w], BF16, tag=f"g1{bid}", name=f"g1{bid}")
                    for fc in range(4):
                        p = psP.tile([128, w], F32, tag=ptag, name=f"pg1_{bid}_{fc}")
                        for kc in range(2):
                            nc.tensor.matmul(
                                p, w2a_b[:, 512 * kc + 128 * fc:512 * kc + 128 * fc + 128],
                                nf1_b[:, w * kc:w * kc + w],
                                start=(kc == 0), stop=(kc == 1))
                        nc.vector.scalar_tensor_tensor(
                            g1[:, w * fc:w * fc + w], p, b2a_c[:, fc:fc + 1],
                            zeros_b8[:, 0:w], op0=mybir.AluOpType.add,
                            op1=mybir.AluOpType.max)
                    st["g1"] = g1
                elif piece == 2:
                    g1 = st["g1"]
                    w2b_b = p2w["w2b"]
                    g2 = wp.tile([128, 4 * w], BF16, tag=f"g2{bid}", name=f"g2{bid}")
                    for fc in range(4):
                        p = psP.tile([128, w], F32, tag=ptag, name=f"pg2_{bid}_{fc}")
                        for kc in range(4):
                            nc.tensor.matmul(
                                p, w2b_b[:, 512 * kc + 128 * fc:512 * kc + 128 * fc + 128],
                                g1[:, w * kc:w * kc + w],
                                start=(kc == 0), stop=(kc == 3))
                        nc.vector.scalar_tensor_tensor(
                            g2[:, w * fc:w * fc + w], p, b2b_c[:, fc:fc + 1],
                            zeros_b8[:, 0:w], op0=mybir.AluOpType.add,
                            op1=mybir.AluOpType.max)
                    st["g2"] = g2
                elif piece == 3:
                    g2, nf1 = st["g2"], st["nf1"]
                    w2c_b = p2w["w2c"]
                    upd2 = wp.tile([128, 2 * w], F32, tag=f"upd2{bid}",
                                   name=f"upd2{bid}")
                    for fo in range(2):
                        p = psP.tile([128, w], F32, tag=ptag, name=f"pu2_{bid}_{fo}")
                        for kc in range(4):
                            nc.tensor.matmul(
                                p, w2c_b[:, 256 * kc + 128 * fo:256 * kc + 128 * fo + 128],
                                g2[:, w * kc:w * kc + w],
                                start=(kc == 0), stop=(kc == 3))
                        nc.vector.scalar_tensor_tensor(
                            upd2[:, w * fo:w * fo + w], p, b2c_c[:, fo:fo + 1],
                            zeros_f[:, 0:w], op0=mybir.AluOpType.add,
                            op1=mybir.AluOpType.add)
                    x2 = wp.tile([128, 2 * w], F32, tag=f"x2{bid}", name=f"x2{bid}")
                    tmpw2 = wp.tile([128, 2 * w], F32, tag=f"updm2{bid}",
                                    name=f"updm2{bid}")
                    nc.vector.tensor_tensor(
                        tmpw2.rearrange("p (f n) -> p f n", f=2),
                        upd2.rearrange("p (f n) -> p f n", f=2), mb2,
                        op=mybir.AluOpType.mult)
                    nc.vector.tensor_tensor(x2, tmpw2, nf1, op=mybir.AluOpType.add)
                    outT = wp.tile([128, 2 * w], F32, tag=f"outT{bid}",
                                   name=f"outT{bid}")
                    block_ln_h(bid, w, x2, ln2g_c, ln2b_c, outT)
                    st["outT"] = outT
                else:
                    outT = st["outT"]
                    o_sb = outp.tile([w, 256], F32, tag="osb", name=f"osb{bid}")
                    for fo in range(2):
                        p_tr = psP.tile([w, 128], F32, tag=ptag,
                                        name=f"ptr_{bid}_{fo}")
                        nc.tensor.transpose(p_tr, outT[:, w * fo:w * fo + w], ident)
                        nc.vector.tensor_copy(o_sb[:, 128 * fo:128 * fo + 128], p_tr)
                    nc.sync.dma_start(p_out[no:no + w, :], o_sb)

            # full 128-node blocks for r=0,1 (fewer mid-loop ops); the last
            # block is split into 64-node halves so only half trails the loop
            P2BLOCKS = (("H0", 0, 0, 64, 10), ("H1", 0, 1, 64, 18),
                        ("H2", 1, 0, 64, 26), ("H3", 1, 1, 64, 34),
                        ("H4", 2, 0, 64, 42), ("H5", 2, 1, 64, 50))

            # phase 0 emission, r=0-first so the pipeline starts early
            geom_block(0)
            afp_chain()
            na_block(0)
            stageA(0)
            geom_block(1)
            na_block(1)
            stageA(1)
            geom_block(2)
            na_block(2)
            h1_live = {}
            for c in range(NC_CHUNKS + 1):
                if c == 2:
                    load_phase2_weights()
                if c % 4 == 0 and c // 4 + 2 < 12:
                    stageA(c // 4 + 2)
                if c < NC_CHUNKS:
                    h1_live[c] = stageB(c)
                if 0 <= c - 1 < NC_CHUNKS:
                    stageC(c - 1, h1_live.pop(c - 1))
                for bid, r, hf, w, base in P2BLOCKS:
                    if base <= c < base + 10 and (c - base) % 2 == 0 and c < 48:
                        phase2_blk(bid, r, hf, w, (c - base) // 2)
                if c == 48:
                    for bid, r, hf, w, base in P2BLOCKS:
                        for k in range(5):
                            if base + 2 * k >= 48:
                                phase2_blk(bid, r, hf, w, k, late=True)

    nc.compile()
    return nc


_NC = None
_CONSTS = _consts()


def build_in_maps(inputs):
    shared = dict(_CONSTS)
    shared.update(_pack_weights(inputs))
    shared["c_biases"] = _pack_biases(inputs)
    shared["c_lnrows"] = _pack_lnrows(inputs)

    node_x = np.asarray(inputs["node_x"], np.float32)
    anchor_x = np.asarray(inputs["anchor_x"], np.float32)
    node_features = np.asarray(inputs["node_features"], np.float32)
    anchor_features = np.asarray(inputs["anchor_features"], np.float32)
    node_mask = np.asarray(inputs["node_mask"], np.float32)

    in_maps = []
    for b in range(B):
        m = dict(shared)
        m["node_x"] = np.ascontiguousarray(node_x[b * N:(b + 1) * N])
        m["anchor_x"] = np.ascontiguousarray(anchor_x[b * A:(b + 1) * A])
        m["node_features"] = np.ascontiguousarray(node_features[b * N:(b + 1) * N])
        m["anchor_features"] = np.ascontiguousarray(
            anchor_features[b * A:(b + 1) * A])
        m["node_mask"] = np.ascontiguousarray(node_mask[b * N:(b + 1) * N])
        in_maps.append(m)
    return in_maps


def kernel(**inputs):
    global _NC
    if _NC is None:
        _NC = _build()
    in_maps = build_in_maps(inputs)
    res = run_bass_kernel_spmd(_NC, in_maps, core_ids=list(range(B)))
    return np.concatenate([res.results[b]["out"] for b in range(B)], axis=0)


# revision 51
# speedup vs baseline: 1.0082x; 1.0003x over previous
"""AnchorSet2NodeMPNN Trainium2 kernel (8 NeuronCores, graph-parallel).

Each core handles one graph (N=384 nodes, A=64 anchors, H=256, E=64).

v2: fp8 DoubleRow main loop + PE-fused anchor-sum  (~282us vs 427us v1).
  Pair columns are ordered a-major (col = 8*a + n within a chunk) so the
  L3 matmul can anchor-sum in PSUM via a stride-0 output AP (same psum
  word revisited every 8 columns; back-to-back same-address accumulation
  is broken on HW, 8-apart is fine).
  d^2[n,a] = |nx|^2 + |ax|^2 - 2 nx.ax           (rank-5 K=5 matmul)
  t'[n,a]  = exp(0.5*ln(relu(d^2)*s))            (ACT-only; avoids the
    sqrt table so the whole kernel uses ONE activation table set --
    see the get_activation_tables override above; staged to DRAM
    chunk-major so rbf gathers are contiguous broadcasts)
  Layer 1 factored: pair @ W1a = nf@W1a[:H] + af@W1a[H:2H] + rbf@W1a[2H:]
    folded into fp8 DoubleRow matmuls: psum = na.T@ind_n (+) [w1r;AF'].T@[rbf;ind_a]
  Layer 2 per-pair fp8 DoubleRow (K=512 as 2 instrs of 2 k-tiles).
  Layer 3 fused with the anchor-mean on the PE: upd psum accumulates all
    64 anchor columns at the same address (stride-0 out AP, start=False
    onto a zeroed region; the zeroing matmul stays on the PE queue).
  LayerNorms + node MLP run feature-transposed (LN stats via gpsimd
  partition allreduce, rstd = exp(-0.5*ln(v))); final PE-transpose back.

Weights are host-prepacked (fp8e4 / bf16) so no on-device weight casts.
fp8 DoubleRow = 2x PE throughput (2 k-tiles/instr at 1 col/cycle); the
cost model's 0.5 cyc/row is optimistic.  Extending fp8 to the small node
MLP triggered chip activity-throttling and was reverted.
fp8 everywhere in the per-pair path gives ~3.2e-3 relative error.
"""
import numpy as np
import ml_dtypes

import concourse.bass as bass
import concourse.mybir as mybir
import concourse.tile as tile
from concourse import bacc
from concourse.bass_utils import run_bass_kernel_spmd

# All activations used here (exp/ln/relu/square/identity/copy) live in the
# "natural_log_exp_and_others" table set, but the table-load pass assigns
# each function its first-containing set, thrashing ACT_TABLE_LOADs (1.3us
# each) between exp_and_others and natural_log_exp_and_others. Pin every
# activation to the one covering set so exactly one load is emitted.
_orig_get_tables = bacc.get_activation_tables
def _single_table_set(arch):
    tabs = _orig_get_tables(arch)
    keep = "natural_log_exp_and_others"
    return {k: (v if k == keep else set()) for k, v in tabs.items()}
bacc.get_activation_tables = _single_table_set

F32 = mybir.dt.float32
BF16 = mybir.dt.bfloat16
F8 = mybir.dt.float8e4
AF = mybir.ActivationFunctionType
DR = mybir.MatmulPerfMode.DoubleRow
F8NP = ml_dtypes.float8_e4m3
BFNP = ml_dtypes.bfloat16

B, N, A, H, E = 8, 384, 64, 256, 64
RBF_D_MAX = 20.0
SIGMA = RBF_D_MAX / E                    # 0.3125
MU = np.linspace(0.0, RBF_D_MAX, E).astype(np.float32)
NC_CHUNKS = 48                           # chunks of 8 nodes x 64 anchors
D2_SCALE = 0.01 / SIGMA**2               # t' = sqrt(d2 * D2_SCALE) = (d/10)/sigma

MAST_COLS = 8192
SLOT0 = MAST_COLS                        # rbf slot region start in rbm
NSLOT = 6


def _consts():
    # a-major pair order: col j = 8*a + n (n in 0..7, a in 0..63)
    # anchor indicator, two 512-col chunk halves per slot
    mast_a = np.zeros((64, 1024), np.float32)
    for j in range(1024):
        mast_a[(j % 512) // 8, j] = 1.0
    # node indicator: mast_n[k, 4096q+512s+j] = 1 iff k == 64q+8s+(j%8)
    mast_n = np.zeros((128, 8192), np.float32)
    for q in range(2):
        for s in range(8):
            for j in range(512):
                mast_n[64 * q + 8 * s + j % 8, 4096 * q + 512 * s + j] = 1.0
    ident = np.eye(128, dtype=np.float32)
    pairm = np.zeros((128, 128), np.float32)
    for j in range(128):
        for m in range(128):
            if j % 64 == m % 64:
                pairm[j, m] = 1.0
    negmusig = np.tile(-(MU / SIGMA).astype(np.float32), 2)
    ones64 = np.ones((1, 64), np.float32)
    return dict(
        c_mast_a=mast_a.astype(F8NP),
        c_mast_n=mast_n.astype(F8NP),
        c_ident=ident,
        c_pairm=pairm,
        c_negmusig=negmusig,
        c_ones64=ones64.astype(BFNP),
    )


def _pack_lnrows(inputs):
    """[128, 512] f32: col 128k+p, row j = v[128*(j//64)+p] for the
    transposed-layout LN affine (k: ln1_g, ln1_b, ln2_g, ln2_b)."""
    out = np.empty((128, 512), np.float32)
    for k, nm in enumerate(("ln1_g", "ln1_b", "ln2_g", "ln2_b")):
        v = np.asarray(inputs[nm], np.float32)
        out[0:64, 128 * k:128 * k + 128] = v[None, 0:128]
        out[64:128, 128 * k:128 * k + 128] = v[None, 128:256]
    return np.ascontiguousarray(out)


def _pack_biases(inputs):
    """Column-pack per-feature vectors: [128, 24] f32, layout-only."""
    cols = []
    for k, n in (("b1b", 4), ("b1c", 2), ("b2a", 4), ("b2b", 4), ("b2c", 2),
                 ("ln1_g", 2), ("ln1_b", 2), ("ln2_g", 2), ("ln2_b", 2)):
        v = np.asarray(inputs[k], np.float32)
        cols.append(v.reshape(n, 128).T)
    return np.ascontiguousarray(np.concatenate(cols, axis=1))


def _kpack(w, kt, dt):
    """[K, F] -> [128, kt*F] with col = F*kc + f, row p = k % 128."""
    K, F = w.shape
    assert K == 128 * kt
    out = np.empty((128, kt * F), np.float32)
    for kc in range(kt):
        out[:, F * kc:F * kc + F] = w[128 * kc:128 * kc + 128, :]
    return np.ascontiguousarray(out).astype(dt)


def _pack_weights(inputs):
    W1a = np.asarray(inputs["W1a"], np.float32)
    d = dict(
        w1nf_bf=_kpack(W1a[0:256], 2, BFNP),       # [128,1024]
        w1af_bf=_kpack(W1a[256:512], 2, BFNP),     # [128,1024]
        w1r8=np.ascontiguousarray(W1a[512:576]).astype(F8NP),   # [64,512]
        w1b8=_kpack(np.asarray(inputs["W1b"], np.float32), 4, F8NP),  # [128,2048]
        w1c8=_kpack(np.asarray(inputs["W1c"], np.float32), 4, F8NP),  # [128,1024]
        w2a_bf=_kpack(np.asarray(inputs["W2a"], np.float32), 2, BFNP),
        w2b_bf=_kpack(np.asarray(inputs["W2b"], np.float32), 4, BFNP),
        w2c_bf=_kpack(np.asarray(inputs["W2c"], np.float32), 4, BFNP),
        b1a_bf=np.asarray(inputs["b1a"], np.float32).reshape(1, 512).astype(BFNP),
    )
    return d


def _build():
    nc = bacc.Bacc("TRN2", target_bir_lowering=False, debug=False)

    # ---- parameters ----
    p_nx = nc.declare_dram_parameter("node_x", [N, 3], F32, isOutput=False)
    p_ax = nc.declare_dram_parameter("anchor_x", [A, 3], F32, isOutput=False)
    p_nf = nc.declare_dram_parameter("node_features", [N, H], F32, isOutput=False)
    p_af = nc.declare_dram_parameter("anchor_features", [A, H], F32, isOutput=False)
    p_mask = nc.declare_dram_parameter("node_mask", [N], F32, isOutput=False)
    p_w1nf = nc.declare_dram_parameter("w1nf_bf", [128, 1024], BF16, isOutput=False)
    p_w1af = nc.declare_dram_parameter("w1af_bf", [128, 1024], BF16, isOutput=False)
    p_w1r8 = nc.declare_dram_parameter("w1r8", [64, 512], F8, isOutput=False)
    p_w1b8 = nc.declare_dram_parameter("w1b8", [128, 2048], F8, isOutput=False)
    p_w1c8 = nc.declare_dram_parameter("w1c8", [128, 1024], F8, isOutput=False)
    p_w2a = nc.declare_dram_parameter("w2a_bf", [128, 1024], BF16, isOutput=False)
    p_w2b = nc.declare_dram_parameter("w2b_bf", [128, 2048], BF16, isOutput=False)
    p_w2c = nc.declare_dram_parameter("w2c_bf", [128, 1024], BF16, isOutput=False)
    p_b1a = nc.declare_dram_parameter("b1a_bf", [1, 512], BF16, isOutput=False)
    c_mast_a = nc.declare_dram_parameter("c_mast_a", [64, 1024], F8, isOutput=False)
    c_mast_n = nc.declare_dram_parameter("c_mast_n", [128, 8192], F8, isOutput=False)
    c_biases = nc.declare_dram_parameter("c_biases", [128, 24], F32, isOutput=False)
    c_lnrows = nc.declare_dram_parameter("c_lnrows", [128, 512], F32, isOutput=False)
    c_ident = nc.declare_dram_parameter("c_ident", [128, 128], F32, isOutput=False)
    c_pairm = nc.declare_dram_parameter("c_pairm", [128, 128], F32, isOutput=False)
    c_negmusig = nc.declare_dram_parameter("c_negmusig", [2 * E], F32, isOutput=False)
    c_ones64 = nc.declare_dram_parameter("c_ones64", [1, 64], BF16, isOutput=False)
    p_out = nc.declare_dram_parameter("out", [N, H], F32, isOutput=True)

    # t staged chunk-major: element (c, a, n) at 512c + 8a + n (a-major pairs)
    t_dram = [nc.dram_tensor(f"t_scratch{r}", [16, 512], F32) for r in range(3)]

    with tile.TileContext(nc) as tc:
        with (
            tc.tile_pool(name="wp", bufs=1) as wp,
            tc.tile_pool(name="psA", bufs=2, space="PSUM") as psA,
            tc.tile_pool(name="psB", bufs=3, space="PSUM") as psB,
            tc.tile_pool(name="psC", bufs=1, space="PSUM") as psC,
            tc.tile_pool(name="tbp", bufs=3) as tbp,
            tc.tile_pool(name="qp", bufs=3) as qp,
            tc.tile_pool(name="h1p", bufs=3) as h1p,
            tc.tile_pool(name="h2p", bufs=3) as h2p,
            tc.tile_pool(name="outp", bufs=2) as outp,
        ):
            dma = nc.sync.dma_start

            # ================= phase 0: loads =================
            nx_sb = [wp.tile([128, 3], F32, tag=f"nx{r}", name=f"nx{r}")
                     for r in range(3)]
            for r in range(3):
                dma(nx_sb[r], p_nx[128 * r:128 * r + 128, :])
            ax_sb = wp.tile([64, 3], F32)
            dma(ax_sb, p_ax[:])
            ident = wp.tile([128, 128], F32); dma(ident, c_ident[:])
            pairm = wp.tile([128, 128], F32)
            nc.scalar.dma_start(pairm, c_pairm[:])
            negmu = wp.tile([128, 1], F32)
            dma(negmu, c_negmusig[:].rearrange("(p o) -> p o", o=1))
            w1nf = wp.tile([128, 1024], BF16); dma(w1nf, p_w1nf[:])
            w1af = wp.tile([128, 1024], BF16); dma(w1af, p_w1af[:])
            nf_sb = [wp.tile([128, 256], F32, tag=f"nfsb{r}", name=f"nfsb{r}")
                     for r in range(3)]
            for r in range(3):
                nc.scalar.dma_start(nf_sb[r], p_nf[128 * r:128 * r + 128, :])

            # one big fp8 tile: node masters + rbf slots (cross-region DR APs)
            rbm = wp.tile([128, MAST_COLS + NSLOT * 1024], F8, name="rbm")
            nc.gpsimd.dma_start(rbm[:, 0:4096], c_mast_n[:, 0:4096])
            nc.gpsimd.dma_start(rbm[:, 4096:8192], c_mast_n[:, 4096:8192])
            for i in range(NSLOT):
                nc.scalar.dma_start(
                    rbm[64:128, SLOT0 + 1024 * i:SLOT0 + 1024 * i + 1024],
                    c_mast_a[:])

            # L1 lhsT tile: na (3 r-blocks) then w1raf
            l1w = wp.tile([128, 2048], F8, name="l1w")
            dma(l1w[0:64, 1536:2048], p_w1r8[:])

            w1b8 = wp.tile([128, 2048], F8)
            nc.gpsimd.dma_start(w1b8, p_w1b8[:])
            w1c8 = wp.tile([128, 1024], F8)
            nc.gpsimd.dma_start(w1c8, p_w1c8[:])
            b1a_rb = wp.tile([1, 512], BF16); dma(b1a_rb, p_b1a[:])
            ones64 = wp.tile([1, 64], BF16); dma(ones64, c_ones64[:])
            af_sb = wp.tile([64, 256], F32)
            nc.scalar.dma_start(af_sb, p_af[:])

            # packed bias columns
            bias_pack = wp.tile([128, 24], F32)
            dma(bias_pack, c_biases[:])
            lnrows = wp.tile([128, 512], F32)
            nc.scalar.dma_start(lnrows, c_lnrows[:])
            off = [0]
            def bp(n):
                t = bias_pack[:, off[0]:off[0] + n]
                off[0] += n
                return t
            b1b_c = bp(4); b1c_c = bp(2); b2a_c = bp(4); b2b_c = bp(4)
            b2c_c = bp(2); ln1g_c = bp(2); ln1b_c = bp(2); ln2g_c = bp(2)
            ln2b_c = bp(2)
            eps_c = wp.tile([128, 1], F32)
            nc.vector.memset(eps_c, 1e-5)
            mask_b = wp.tile([128, N], F32)
            mb_src = p_mask[0:1]
            nc.sync.dma_start(
                out=mask_b,
                in_=bass.AP(tensor=mb_src.tensor, offset=0, ap=[[0, 128], [1, N]]),
            )
            zeros_b = wp.tile([128, 512], BF16)
            nc.vector.memset(zeros_b, 0.0)
            zw = wp.tile([128, 128], F8)
            nc.vector.memset(zw, 0.0)

            # ================= phase 0: geometry (defs) =================
            axs = wp.tile([64, 3], F32)
            nc.vector.tensor_tensor(axs, ax_sb, ax_sb, op=mybir.AluOpType.mult)
            aa2 = wp.tile([64, 1], F32)
            nc.vector.reduce_sum(aa2, axs, axis=mybir.AxisListType.X)
            aug_a = wp.tile([64, 5], F32)
            nc.vector.tensor_scalar_mul(aug_a[:, 0:3], ax_sb, -2.0)
            nc.vector.memset(aug_a[:, 3:4], 1.0)
            nc.vector.tensor_copy(aug_a[:, 4:5], aa2)
            p_t = psA.tile([128, 64], F32, tag="a")
            nc.tensor.transpose(p_t[0:5, 0:64], aug_a, ident[0:64, 0:64])
            axaug = wp.tile([5, 64], F32)
            nc.vector.tensor_copy(axaug, p_t[0:5, 0:64])

            nfT_b = wp.tile([128, 768], BF16)   # nf.T bf16, kc-major
            nfT_f = wp.tile([128, 768], F32)    # nf.T f32

            def geom_block(r):
                """distances for node block r -> t_dram[r]; nf.T transposes."""
                nxs = wp.tile([128, 3], F32, tag="nxs")
                nc.vector.tensor_tensor(nxs, nx_sb[r], nx_sb[r], op=mybir.AluOpType.mult)
                nn2 = wp.tile([128, 1], F32, tag="nn2")
                nc.vector.reduce_sum(nn2, nxs, axis=mybir.AxisListType.X)
                aug_n = wp.tile([128, 5], F32, tag="augn")
                nc.vector.tensor_copy(aug_n[:, 0:3], nx_sb[r])
                nc.vector.tensor_copy(aug_n[:, 3:4], nn2)
                nc.vector.memset(aug_n[:, 4:5], 1.0)
                p_tn = psB.tile([128, 128], F32, tag="b")
                nc.tensor.transpose(p_tn[0:5, :], aug_n, ident)
                nxaugT = wp.tile([5, 128], F32, tag="nxaugT")
                nc.vector.tensor_copy(nxaugT, p_tn[0:5, :])
                p_d2 = psA.tile([64, 128], F32, tag="a")
                nc.tensor.matmul(p_d2, axaug, nxaugT, start=True, stop=True)
                # t = sqrt(relu(d2)*s) as exp(0.5*ln(.)): ACT-only chain on
                # the single loaded table set (relu clamps fp slop, ln(0)=-inf
                # exponentiates to t=0, matching the d2<=0 limit)
                d2c = wp.tile([64, 128], F32, tag="d2c")
                nc.scalar.activation(d2c, p_d2, AF.Relu, bias=0.0, scale=D2_SCALE)
                tl = wp.tile([64, 128], F32, tag="tl")
                nc.scalar.activation(tl, d2c, AF.Ln, bias=0.0, scale=1.0)
                t_sb = wp.tile([64, 128], F32, tag="tsb")
                nc.scalar.activation(t_sb, tl, AF.Exp, bias=0.0, scale=0.5)
                dma(
                    out=bass.AP(tensor=t_dram[r][:].tensor, offset=0,
                                ap=[[8, 64], [512, 16], [1, 8]]),
                    in_=t_sb.rearrange("p (c n) -> p c n", c=16, n=8),
                )
                for c in range(2):
                    p_tr = psB.tile([128, 128], F32, tag="b")
                    nc.tensor.transpose(p_tr, nf_sb[r][:, 128 * c:128 * c + 128], ident)
                    nc.vector.tensor_copy(
                        nfT_b[:, 384 * c + 128 * r:384 * c + 128 * r + 128], p_tr)
                    nc.vector.tensor_copy(
                        nfT_f[:, 384 * c + 128 * r:384 * c + 128 * r + 128], p_tr)

            def na_block(r):
                """NA matmuls -> fp8 into l1w na region r."""
                p_na = psB.tile([128, 512], F32, tag="b")
                for kc in range(2):
                    nc.tensor.matmul(
                        p_na,
                        nfT_b[:, 384 * kc + 128 * r:384 * kc + 128 * r + 128],
                        w1nf[:, 512 * kc:512 * kc + 512],
                        start=(kc == 0), stop=(kc == 1),
                    )
                with nc.allow_low_precision(reason="fp8 main-loop operands"):
                    nc.vector.tensor_copy(l1w[:, 512 * r:512 * r + 512], p_na)

            def afp_chain():
                """af.T; AF' = af@W1a[H:2H] + b1a -> fp8 l1w rows 64:128."""
                afT_b = wp.tile([128, 128], BF16)
                for c in range(2):
                    p_tr = psA.tile([128, 64], F32, tag="a")
                    nc.tensor.transpose(p_tr[:, 0:64], af_sb[:, 128 * c:128 * c + 128],
                                        ident[0:64, 0:64])
                    nc.vector.tensor_copy(afT_b[:, 64 * c:64 * c + 64], p_tr[:, 0:64])
                p_af2 = psB.tile([64, 512], F32, tag="b")
                for kc in range(2):
                    nc.tensor.matmul(p_af2, afT_b[:, 64 * kc:64 * kc + 64],
                                     w1af[:, 512 * kc:512 * kc + 512],
                                     start=(kc == 0), stop=False)
                nc.tensor.matmul(p_af2, ones64, b1a_rb, start=False, stop=True)
                with nc.allow_low_precision(reason="fp8 main-loop operands"):
                    nc.vector.tensor_copy(l1w[64:128, 1536:2048], p_af2)

            updT = [wp.tile([128, 256], F32, tag=f"updT{r}", name=f"updT{r}")
                    for r in range(3)]

            def ap3(t, offset, d1, n1, d2_, n2):
                return bass.AP(tensor=t.tensor, offset=t.offset + offset,
                               ap=[list(t.ap[0]), [d1, n1], [d2_, n2]])

            # ================= main loop (software-pipelined) =================
            def stageA(pp):
                """t gather + rbf for superchunk pair (2pp, 2pp+1).

                tb rows 0:64 = sc 2pp pairs (a-major), rows 64:128 = sc 2pp+1;
                Exp writes fp8 straight into rbm slots (rows 0:64)."""
                tb = tbp.tile([128, 1024], F32, tag="tb", name=f"tb{pp}")
                for h in range(2):
                    sc = 2 * pp + h
                    cl = 2 * sc - 16 * (sc // 8)
                    nc.sync.dma_start(
                        out=tb[64 * h:64 * h + 64, :],
                        in_=bass.AP(tensor=t_dram[sc // 8][:].tensor,
                                    offset=512 * cl,
                                    ap=[[0, 64], [1, 1024]]),
                    )
                qx = qp.tile([128, 1024], F32, tag="qx", name=f"qx{pp}")
                nc.scalar.activation(qx, tb, AF.Square, bias=negmu[:, 0:1], scale=1.0)
                for h in range(2):
                    slot = (2 * pp + h) % NSLOT
                    nc.scalar.activation(
                        rbm[0:64, SLOT0 + 1024 * slot:SLOT0 + 1024 * slot + 1024],
                        qx[64 * h:64 * h + 64, :], AF.Exp, bias=0.0, scale=-1.0)

            def stageB(c):
                """L1 fp8 DoubleRow matmuls + relu -> h1 (fp8)."""
                q = (c // 8) % 2
                s = c % 8
                r = c // 16
                mast_off = 4096 * q + 512 * s
                rt_off = SLOT0 + 1024 * ((c // 2) % NSLOT) + 512 * (c % 2)
                rhs = ap3(rbm, mast_off, rt_off - mast_off, 2, 1, 512)
                h1 = h1p.tile([128, 2048], F8, tag="h1", name=f"h1_{c}")
                for hh in range(2):
                    p1 = psA.tile([128, 1024], F32, tag="a", name=f"p1_{c}_{hh}")
                    for i in range(2):
                        fc = 2 * hh + i
                        lhsT = ap3(l1w, 512 * r + 128 * fc, 1536 - 512 * r, 2, 1, 128)
                        nc.tensor.matmul(p1[:, 512 * i:512 * i + 512], lhsT, rhs,
                                         start=True, stop=True, perf_mode=DR)
                    with nc.allow_low_precision(reason="fp8 main-loop operands"):
                        nc.scalar.activation(h1[:, 1024 * hh:1024 * hh + 1024],
                                             p1, AF.Relu, bias=0.0, scale=1.0)
                return h1

            def stageC(c, h1):
                """L2 fp8 DR + relu+bias -> h2s (fp8); L3 fp8 DR with PE
                anchor-sum (stride-0 psum accumulate) -> updT."""
                h2s = h2p.tile([128, 2048], F8, tag="h2", name=f"h2_{c}")
                for fc in range(4):
                    p2 = psB.tile([128, 512], F32, tag="b", name=f"p2_{c}_{fc}")
                    for kp in range(2):
                        lhsT = ap3(w1b8, 1024 * kp + 128 * fc, 512, 2, 1, 128)
                        rhs = ap3(h1, 1024 * kp, 512, 2, 1, 512)
                        nc.tensor.matmul(p2, lhsT, rhs, start=(kp == 0),
                                         stop=(kp == 1), perf_mode=DR)
                    with nc.allow_low_precision(reason="fp8 main-loop operands"):
                        nc.vector.scalar_tensor_tensor(
                            h2s[:, 512 * fc:512 * fc + 512], p2,
                            b1b_c[:, fc:fc + 1], zeros_b,
                            op0=mybir.AluOpType.add, op1=mybir.AluOpType.max)
                # L3 + anchor-sum: U[fo*8+n] += sum_a sum_k w1c[k,fo]h2[k,8a+n]
                U = psB.tile([128, 16], F32, tag="b", name=f"U_{c}")
                nc.tensor.matmul(U, zw, zw[:, 0:16], start=True, stop=True)
                for fo in range(2):
                    out2 = bass.AP(tensor=U.tensor, offset=U.offset + 8 * fo,
                                   ap=[list(U.ap[0]), [0, 64], [1, 8]])
                    for kp in range(2):
                        lhsT = ap3(w1c8, 512 * kp + 128 * fo, 256, 2, 1, 128)
                        rhs = ap3(h2s, 1024 * kp, 512, 2, 1, 512)
                        nc.tensor.matmul(out2, lhsT, rhs, start=False,
                                         stop=(fo == 1 and kp == 1),
                                         perf_mode=DR, skip_group_check=True)
                r = c // 16
                dst = bass.AP(tensor=updT[r].tensor,
                              offset=updT[r].offset + 8 * (c % 16),
                              ap=[list(updT[r].ap[0]), [128, 2], [1, 8]])
                src = bass.AP(tensor=U.tensor, offset=U.offset,
                              ap=[list(U.ap[0]), [8, 2], [1, 8]])
                nc.vector.tensor_copy(dst, src)

            # ====== phase 2 (node path), per-128-node block, overlapped ======
            zeros_f = wp.tile([128, 128], F32)
            nc.vector.memset(zeros_f, 0.0)
            zeros_b8 = wp.tile([128, 128], BF16)
            nc.vector.memset(zeros_b8, 0.0)
            p2w = {}

            def load_phase2_weights():
                for nm, prm, shp in (("w2a", p_w2a, [128, 1024]),
                                     ("w2b", p_w2b, [128, 2048]),
                                     ("w2c", p_w2c, [128, 1024])):
                    t = wp.tile(shp, BF16, name=nm, tag=nm)
                    dma(t, prm[:])
                    p2w[nm] = t

            def block_ln_h(bid, w, x_in, g_c, b_c, out_t):
                """LN over 256 feats for w nodes; x_in/out_t [128, 2w]
                compact fo-major (col = w*fo + n)."""
                x3 = x_in.rearrange("p (f n) -> p f n", f=2)
                red = wp.tile([128, 2 * w], F32, tag=f"lnr{bid}", name=f"lnr{bid}")
                nc.gpsimd.partition_all_reduce(
                    red, x_in, channels=128, reduce_op=bass.bass_isa.ReduceOp.add)
                Ssum = wp.tile([128, w], F32, tag=f"lnS{bid}", name=f"lnS{bid}")
                nc.vector.tensor_tensor(Ssum, red[:, 0:w], red[:, w:2 * w],
                                        op=mybir.AluOpType.add)
                Sb = bass.AP(tensor=Ssum.tensor, offset=Ssum.offset,
                             ap=[list(Ssum.ap[0]), [0, 2], list(Ssum.ap[1])])
                xc = wp.tile([128, 2 * w], F32, tag=f"lnxc{bid}", name=f"lnxc{bid}")
                nc.vector.scalar_tensor_tensor(
                    xc.rearrange("p (f n) -> p f n", f=2), Sb, -1.0 / 256.0, x3,
                    op0=mybir.AluOpType.mult, op1=mybir.AluOpType.add)
                sq = wp.tile([128, 2 * w], F32, tag=f"lnsq{bid}", name=f"lnsq{bid}")
                nc.vector.tensor_tensor(sq, xc, xc, op=mybir.AluOpType.mult)
                red2 = wp.tile([128, 2 * w], F32, tag=f"lnr2{bid}", name=f"lnr2{bid}")
                nc.gpsimd.partition_all_reduce(
                    red2, sq, channels=128, reduce_op=bass.bass_isa.ReduceOp.add)
                V = wp.tile([128, w], F32, tag=f"lnV{bid}", name=f"lnV{bid}")
                nc.vector.tensor_tensor(V, red2[:, 0:w], red2[:, w:2 * w],
                                        op=mybir.AluOpType.add)
                sd = wp.tile([128, w], F32, tag=f"lnsd{bid}", name=f"lnsd{bid}")
                nc.scalar.activation(sd, V, AF.Ln, bias=eps_c[:, 0:1],
                                     scale=1.0 / 256.0)
                rstd = wp.tile([128, w], F32, tag=f"lnrstd{bid}", name=f"lnrstd{bid}")
                nc.scalar.activation(rstd, sd, AF.Exp, bias=0.0, scale=-0.5)
                rb = bass.AP(tensor=rstd.tensor, offset=rstd.offset,
                             ap=[list(rstd.ap[0]), [0, 2], list(rstd.ap[1])])
                y = wp.tile([128, 2 * w], F32, tag=f"lny{bid}", name=f"lny{bid}")
                nc.vector.tensor_tensor(y.rearrange("p (f n) -> p f n", f=2),
                                        xc.rearrange("p (f n) -> p f n", f=2), rb,
                                        op=mybir.AluOpType.mult)
                for fo in range(2):
                    nc.scalar.activation(out_t[:, w * fo:w * fo + w],
                                         y[:, w * fo:w * fo + w],
                                         AF.Identity, bias=b_c[:, fo:fo + 1],
                                         scale=g_c[:, fo:fo + 1])

            p2state = {}

            def phase2_blk(bid, r, hf, w, piece, late=False):
                """phase 2 on a w-node block (nodes 128r + w*hf ..+w);
                intermediate tiles compact (col = w*fo + n or w*fc + n)."""
                no = 128 * r + w * hf
                psP = psB if late else psC
                ptag = "b" if late else "c"
                st = p2state.setdefault(bid, {})
                mb = mask_b[:, no:no + w]
                mb2 = bass.AP(tensor=mb.tensor, offset=mb.offset,
                              ap=[list(mb.ap[0]), [0, 2], list(mb.ap[1])])
                if piece == 0:
                    # upd = updT/64 + b1c; x1 + LN1 (+ bf16 cast)
                    upd = wp.tile([128, 2 * w], F32, tag=f"upd{bid}", name=f"upd{bid}")
                    for fo in range(2):
                        nc.scalar.activation(
                            upd[:, w * fo:w * fo + w],
                            updT[r][:, 128 * fo + w * hf:128 * fo + w * hf + w],
                            AF.Identity, bias=b1c_c[:, fo:fo + 1], scale=1.0 / 64.0)
                    x1 = wp.tile([128, 2 * w], F32, tag=f"x1{bid}", name=f"x1{bid}")
                    tmpw = wp.tile([128, 2 * w], F32, tag=f"updm{bid}", name=f"updm{bid}")
                    nc.vector.tensor_tensor(
                        tmpw.rearrange("p (f n) -> p f n", f=2),
                        upd.rearrange("p (f n) -> p f n", f=2), mb2,
                        op=mybir.AluOpType.mult)
                    nfs = nfT_f[:, no:no + w]
                    nfv = bass.AP(tensor=nfs.tensor, offset=nfs.offset,
                                  ap=[list(nfs.ap[0]), [384, 2], list(nfs.ap[1])])
                    nc.vector.tensor_tensor(
                        x1.rearrange("p (f n) -> p f n", f=2),
                        tmpw.rearrange("p (f n) -> p f n", f=2), nfv,
                        op=mybir.AluOpType.add)
                    nf1 = wp.tile([128, 2 * w], F32, tag=f"nf1{bid}", name=f"nf1{bid}")
                    block_ln_h(bid, w, x1, ln1g_c, ln1b_c, nf1)
                    nf1_b = wp.tile([128, 2 * w], BF16, tag=f"nf1b{bid}",
                                    name=f"nf1b{bid}")
                    nc.vector.tensor_copy(nf1_b, nf1)
                    st["nf1"], st["nf1_b"] = nf1, nf1_b
                elif piece == 1:
                    nf1_b = st["nf1_b"]
                    w2a_b = p2w["w2a"]
                    g1 = wp.tile([128, 4 * w], BF16, tag=f"g1{bid}", name=f"g1{bid}")
                    for fc in range(4):
                        p = psP.tile([128, w], F32, tag=ptag, name=f"pg1_{bid}_{fc}")
                        for kc in range(2):
                            nc.tensor.matmul(
                                p, w2a_b[:, 512 * kc + 128 * fc:512 * kc + 128 * fc + 128],
                                nf1_b[:, w * kc:w * kc + w],
                                start=(kc == 0), stop=(kc == 1))
                        nc.vector.scalar_tensor_tensor(
                            g1[:, w * fc:w * fc + w], p, b2a_c[:, fc:fc + 1],
                            zeros_b8[:, 0:w], op0=mybir.AluOpType.add,
                            op1=mybir.AluOpType.max)
                    st["g1"] = g1
                elif piece == 2:
                    g1 = st["g1"]
                    w2b_b = p2w["w2b"]
                    g2 = wp.tile([128, 4 * w], BF16, tag=f"g2{bid}", name=f"g2{bid}")
                    for fc in range(4):
                        p = psP.tile([128, w], F32, tag=ptag, name=f"pg2_{bid}_{fc}")
                        for kc in range(4):
                            nc.tensor.matmul(
                                p, w2b_b[:, 512 * kc + 128 * fc:512 * kc + 128 * fc + 128],
                                g1[:, w * kc:w * kc + w],
                                start=(kc == 0), stop=(kc == 3))
                        nc.vector.scalar_tensor_tensor(
                            g2[:, w * fc:w * fc + w], p, b2b_c[:, fc:fc + 1],
                            zeros_b8[:, 0:w], op0=mybir.AluOpType.add,
                            op1=mybir.AluOpType.max)
                    st["g2"] = g2
                elif piece == 3:
                    g2, nf1 = st["g2"], st["nf1"]
                    w2c_b = p2w["w2c"]
                    upd2 = wp.tile([128, 2 * w], F32, tag=f"upd2{bid}",
                                   name=f"upd2{bid}")
                    for fo in range(2):
                        p = psP.tile([128, w], F32, tag=ptag, name=f"pu2_{bid}_{fo}")
                        for kc in range(4):
                            nc.tensor.matmul(
                                p, w2c_b[:, 256 * kc + 128 * fo:256 * kc + 128 * fo + 128],
                                g2[:, w * kc:w * kc + w],
                                start=(kc == 0), stop=(kc == 3))
                        nc.vector.scalar_tensor_tensor(
                            upd2[:, w * fo:w * fo + w], p, b2c_c[:, fo:fo + 1],
                            zeros_f[:, 0:w], op0=mybir.AluOpType.add,
                            op1=mybir.AluOpType.add)
                    x2 = wp.tile([128, 2 * w], F32, tag=f"x2{bid}", name=f"x2{bid}")
                    tmpw2 = wp.tile([128, 2 * w], F32, tag=f"updm2{bid}",
                                    name=f"updm2{bid}")
                    nc.vector.tensor_tensor(
                        tmpw2.rearrange("p (f n) -> p f n", f=2),
                        upd2.rearrange("p (f n) -> p f n", f=2), mb2,
                        op=mybir.AluOpType.mult)
                    nc.vector.tensor_tensor(x2, tmpw2, nf1, op=mybir.AluOpType.add)
                    outT = wp.tile([128, 2 * w], F32, tag=f"outT{bid}",
                                   name=f"outT{bid}")
                    block_ln_h(bid, w, x2, ln2g_c, ln2b_c, outT)
                    st["outT"] = outT
                else:
                    outT = st["outT"]
                    o_sb = outp.tile([w, 256], F32, tag="osb", name=f"osb{bid}")
                    for fo in range(2):
                        p_tr = psP.tile([w, 128], F32, tag=ptag,
                                        name=f"ptr_{bid}_{fo}")
                        nc.tensor.transpose(p_tr, outT[:, w * fo:w * fo + w], ident)
                        nc.vector.tensor_copy(o_sb[:, 128 * fo:128 * fo + 128], p_tr)
                    nc.sync.dma_start(p_out[no:no + w, :], o_sb)

            # full 128-node blocks for r=0,1 (fewer mid-loop ops); the last
            # block is split into 64-node halves so only half trails the loop
            P2BLOCKS = (("H0", 0, 0, 64, 11), ("H1", 0, 1, 64, 19),
                        ("H2", 1, 0, 64, 27), ("H3", 1, 1, 64, 35),
                        ("H4", 2, 0, 64, 43), ("H5", 2, 1, 64, 51))

            # phase 0 emission, r=0-first so the pipeline starts early
            geom_block(0)
            afp_chain()
            na_block(0)
            stageA(0)
            geom_block(1)
            na_block(1)
            stageA(1)
            geom_block(2)
            na_block(2)
            h1_live = {}
            for c in range(NC_CHUNKS + 1):
                if c == 2:
                    load_phase2_weights()
                if c % 4 == 0 and c // 4 + 2 < 12:
                    stageA(c // 4 + 2)
                if c < NC_CHUNKS:
                    h1_live[c] = stageB(c)
                if 0 <= c - 1 < NC_CHUNKS:
                    stageC(c - 1, h1_live.pop(c - 1))
                for bid, r, hf, w, base in P2BLOCKS:
                    if base <= c < base + 10 and (c - base) % 2 == 0 and c < 48:
                        phase2_blk(bid, r, hf, w, (c - base) // 2)
                if c == 48:
                    for bid, r, hf, w, base in P2BLOCKS:
                        for k in range(5):
                            if base + 2 * k >= 48:
                                phase2_blk(bid, r, hf, w, k, late=True)

    nc.compile()
    return nc


_NC = None
_CONSTS = _consts()


def build_in_maps(inputs):
    shared = dict(_CONSTS)
    shared.update(_pack_weights(inputs))
    shared["c_biases"] = _pack_biases(inputs)
    shared["c_lnrows"] = _pack_lnrows(inputs)

    node_x = np.asarray(inputs["node_x"], np.float32)
    anchor_x = np.asarray(inputs["anchor_x"], np.float32)
    node_features = np.asarray(inputs["node_features"], np.float32)
    anchor_features = np.asarray(inputs["anchor_features"], np.float32)
    node_mask = np.asarray(inputs["node_mask"], np.float32)

    in_maps = []
    for b in range(B):
        m = dict(shared)
        m["node_x"] = np.ascontiguousarray(node_x[b * N:(b + 1) * N])
        m["anchor_x"] = np.ascontiguousarray(anchor_x[b * A:(b + 1) * A])
        m["node_features"] = np.ascontiguousarray(node_features[b * N:(b + 1) * N])
        m["anchor_features"] = np.ascontiguousarray(
            anchor_features[b * A:(b + 1) * A])
        m["node_mask"] = np.ascontiguousarray(node_mask[b * N:(b + 1) * N])
        in_maps.append(m)
    return in_maps


def kernel(**inputs):
    global _NC
    if _NC is None:
        _NC = _build()
    in_maps = build_in_maps(inputs)
    res = run_bass_kernel_spmd(_NC, in_maps, core_ids=list(range(B)))
    return np.concatenate([res.results[b]["out"] for b in range(B)], axis=0)
